# revision 1
# baseline (speedup 1.0000x reference)
"""Content-guided attention kernel for Trainium2, 8 NeuronCores SPMD.

Sharding: 8 cores = (batch b in {0,1}) x (query-chunk qc in {0..3}).
Each core computes 1024 query positions of batch b end-to-end:
q/k/vT projections, 8-head attention over all 3072 keys, o-projection,
residual and LayerNorm.  No collectives needed; host splits/concats.

Per-core layout highlights:
 - scores computed transposed S^T[kpos, qpos] so softmax sum folds into the
   attn@V matmul via a ones-column appended to V^T (no partition reductions)
 - head_dim=32 scores matmuls are packed 4-at-a-time into the PE's 32-row
   groups via tile_position (4x concurrency at K=32)
 - exp split between ScalarE (exact table exp) and VectorE (Schraudolph
   bit-trick exp, ~3% elementwise, ~1.5e-4 end-to-end after softmax
   cancellation + residual/LN dilution)
 - LayerNorm rstd computed as exp(-0.5*ln(var+eps)) to stay inside the
   single natural_log_exp ACT table set (no table switch thrash)
"""

import numpy as np

C = 256
NH = 8
D = 32
NQ_CORE = 1024
NK = 3072
N_CORES = 8
SCALE = float(D) ** -0.5

# Schraudolph exp constants (validated vs reference offline: 3.0% max elem
# rel err on the observed score range; 1.5e-4 absmax on the final output).
_SCHR_A = float(np.float32(SCALE * (1 << 23) / np.log(2.0)))
_SCHR_B = float(np.float32(127.0 * (1 << 23) - 365000.0))

# every 3rd exp slot goes to the vector engine to offload the ACT bottleneck
def _use_dve_exp(slot: int) -> bool:
    return slot % 3 == 2


def _apply_walrus_wait_patch():
    """This walrus build accepts only ONE sync-wait per instruction; split
    extra waits onto single-wait NoOps inserted before the instruction
    (same engine, same block => per-engine program order preserved)."""
    import orjson
    import concourse.bass_utils as bass_utils
    import concourse.bass2jax as bass2jax

    if getattr(bass_utils, "_ant_wait_split_patch", False):
        return
    bass_utils._ant_wait_split_patch = True
    counter = [0]

    def _split_waits(bir_bytes: bytes) -> bytes:
        d = orjson.loads(bir_bytes)
        changed = False

        def process_blocks(blocks):
            nonlocal changed
            for b in blocks:
                insts = b.get("instructions")
                if insts:
                    new = []
                    for ins in insts:
                        si = ins.get("sync_info")
                        waits = si.get("on_wait") if si else None
                        if waits and len(waits) > 1:
                            changed = True
                            for w in waits[:-1]:
                                counter[0] += 1
                                new.append({
                                    "debug": ins.get("debug", 0),
                                    "engine": ins["engine"],
                                    "ins": [],
                                    "outs": [],
                                    "name": f"antwsplit-{counter[0]}",
                                    "opcode": "NoOp",
                                    "sync_info": {"on_wait": [w], "on_update": []},
                                })
                            si["on_wait"] = [waits[-1]]
                        new.append(ins)
                    b["instructions"] = new
                if b.get("blocks"):
                    process_blocks(b["blocks"])

        for f in d.get("functions", []):
            process_blocks(f.get("blocks", []))
        return orjson.dumps(d) if changed else bir_bytes

    orig = bass_utils.compile_bir_kernel

    def compile_bir_kernel(bir, tmpdir, neff_name="file.neff", **kw):
        if isinstance(bir, (bytes, bytearray)):
            bir = _split_waits(bytes(bir))
        elif isinstance(bir, str):
            bir = _split_waits(bir.encode()).decode()
        return orig(bir, tmpdir, neff_name=neff_name, **kw)

    bass_utils.compile_bir_kernel = compile_bir_kernel
    bass2jax.compile_bir_kernel = compile_bir_kernel


def build_program():
    import concourse.bass as bass
    import concourse.tile as tile
    from concourse import mybir

    f32 = mybir.dt.float32
    i32 = mybir.dt.int32
    Alu = mybir.AluOpType
    Act = mybir.ActivationFunctionType

    nc = bass.Bass()

    x_d = nc.dram_tensor("x", [C, NQ_CORE], f32, kind="ExternalInput")
    kv_d = nc.dram_tensor("kv", [C, NK], f32, kind="ExternalInput")
    qwT_d = nc.dram_tensor("qwT", [C, C], f32, kind="ExternalInput")
    kwT_d = nc.dram_tensor("kwT", [C, C], f32, kind="ExternalInput")
    vwT_d = nc.dram_tensor("vwT", [C, C], f32, kind="ExternalInput")
    owT_d = nc.dram_tensor("owT", [C, C], f32, kind="ExternalInput")
    ident_d = nc.dram_tensor("ident", [C, C], f32, kind="ExternalInput")
    qb_d = nc.dram_tensor("qb2", [1, C], f32, kind="ExternalInput")
    kb_d = nc.dram_tensor("kb2", [1, C], f32, kind="ExternalInput")
    vb_d = nc.dram_tensor("vb2", [1, C], f32, kind="ExternalInput")
    ob_d = nc.dram_tensor("ob2", [1, C], f32, kind="ExternalInput")
    lnw_d = nc.dram_tensor("lnw2", [1, C], f32, kind="ExternalInput")
    lnb_d = nc.dram_tensor("lnb2", [1, C], f32, kind="ExternalInput")
    y_d = nc.dram_tensor("y", [NQ_CORE, C], f32, kind="ExternalOutput")

    def bcast_part(ap, n):
        # partition-stride-0 view: replicate one partition row across n
        # (DRAM sources only; SBUF partition dims need nonzero step)
        return bass.AP(tensor=ap.tensor, offset=ap.offset,
                       ap=[[0, n]] + [list(a) for a in ap.ap[1:]])

    def bcast_sbuf_row(ap, n):
        # SBUF [1, F] row -> [n, F] DMA source: keep the 1-partition dim,
        # replicate via a step-0 free dim (legal for DMA reads)
        return bass.AP(tensor=ap.tensor, offset=ap.offset,
                       ap=[list(ap.ap[0]), [0, n]] + [list(a) for a in ap.ap[1:]])

    from contextlib import ExitStack
    with tile.TileContext(nc) as tc, ExitStack() as ctx:
            consts = ctx.enter_context(tc.tile_pool(name="consts", bufs=1))
            data = ctx.enter_context(tc.tile_pool(name="data", bufs=1))
            acts = ctx.enter_context(tc.tile_pool(name="acts", bufs=1))
            # ---- constants ----
            w_sb = {}
            for nm, dt_ in (("qwT", qwT_d), ("kwT", kwT_d), ("vwT", vwT_d),
                            ("owT", owT_d), ("ident", ident_d)):
                t = consts.tile([128, 2, C], f32, tag=f"w_{nm}")
                nc.sync.dma_start(out=t, in_=dt_.rearrange("(a p) c -> p a c", p=128))
                w_sb[nm] = t
            qb_row = consts.tile([1, C], f32, tag="qb_row")
            kb_row = consts.tile([1, C], f32, tag="kb_row")
            vb_row = consts.tile([1, C], f32, tag="vb_row")
            ob_row = consts.tile([1, C], f32, tag="ob_row")
            for t, dt_ in ((qb_row, qb_d), (kb_row, kb_d), (vb_row, vb_d), (ob_row, ob_d)):
                nc.sync.dma_start(out=t, in_=dt_[:])
            lnw_bc = consts.tile([128, C], f32, tag="lnw_bc")
            lnb_bc = consts.tile([128, C], f32, tag="lnb_bc")
            nc.sync.dma_start(out=lnw_bc, in_=bcast_part(lnw_d[:], 128))
            nc.sync.dma_start(out=lnb_bc, in_=bcast_part(lnb_d[:], 128))
            ones_row = consts.tile([1, 512], f32, tag="ones_row")
            nc.vector.memset(ones_row, 1.0)
            eps_col = consts.tile([128, 1], f32, tag="eps_col")
            nc.vector.memset(eps_col, 1e-5)

            # ---- input activations ----
            x_sb = data.tile([128, 2, NQ_CORE], f32, tag="x_sb")
            nc.sync.dma_start(out=x_sb, in_=x_d.rearrange("(a p) n -> p a n", p=128))
            kv_sb = data.tile([128, 2, NK], f32, tag="kv_sb")
            nc.sync.dma_start(out=kv_sb, in_=kv_d.rearrange("(a p) n -> p a n", p=128))

            q_sb = acts.tile([128, 2, NQ_CORE], f32, tag="q_sb")
            k_sb = acts.tile([128, 2, NK], f32, tag="k_sb")
            vT_aug = acts.tile([128, 24, NH, D + 1], f32, tag="vT_aug")
            nc.vector.memset(vT_aug[:, :, :, D:D + 1], 1.0)

            # ---- projections ----
            with tc.tile_pool(name="proj_ps", bufs=4, space="PSUM") as proj_ps:
                # q = qw @ x + qb   (chunks of output channels x 512 cols)
                for mc in range(2):
                    for nb in range(2):
                        ps = proj_ps.tile([128, 512], f32, tag="proj")
                        for kc2 in range(2):
                            nc.tensor.matmul(
                                ps, lhsT=w_sb["qwT"][:, kc2, mc * 128:(mc + 1) * 128],
                                rhs=x_sb[:, kc2, nb * 512:(nb + 1) * 512],
                                start=(kc2 == 0), stop=False)
                        nc.tensor.matmul(
                            ps, lhsT=qb_row[:, mc * 128:(mc + 1) * 128],
                            rhs=ones_row[:, 0:512], start=False, stop=True)
                        nc.vector.tensor_copy(q_sb[:, mc, nb * 512:(nb + 1) * 512], ps)
                # k = kw @ kv + kb
                for mc in range(2):
                    for nb in range(6):
                        ps = proj_ps.tile([128, 512], f32, tag="proj")
                        for kc2 in range(2):
                            nc.tensor.matmul(
                                ps, lhsT=w_sb["kwT"][:, kc2, mc * 128:(mc + 1) * 128],
                                rhs=kv_sb[:, kc2, nb * 512:(nb + 1) * 512],
                                start=(kc2 == 0), stop=False)
                        nc.tensor.matmul(
                            ps, lhsT=kb_row[:, mc * 128:(mc + 1) * 128],
                            rhs=ones_row[:, 0:512], start=False, stop=True)
                        nc.vector.tensor_copy(k_sb[:, mc, nb * 512:(nb + 1) * 512], ps)
                # vT[n, c] = (kv^T @ vw^T)[n, c] + vb[c], written per-head with
                # a ones column appended (softmax denominator trick)
                for nn in range(24):
                    ps = proj_ps.tile([128, C], f32, tag="proj")
                    for kc2 in range(2):
                        nc.tensor.matmul(
                            ps, lhsT=kv_sb[:, kc2, nn * 128:(nn + 1) * 128],
                            rhs=w_sb["vwT"][:, kc2, :], start=(kc2 == 0), stop=False)
                    nc.tensor.matmul(ps, lhsT=ones_row[0:1, 0:128], rhs=vb_row[:],
                                     start=False, stop=True)
                    nc.vector.tensor_copy(
                        vT_aug[:, nn, :, 0:D],
                        ps.rearrange("p (h e) -> p h e", h=NH))

            # ---- attention + o-proj + LN ----
            with tc.tile_pool(name="s_ps", bufs=3, space="PSUM") as s_pool, \
                 tc.tile_pool(name="o_ps", bufs=1, space="PSUM") as o_pool, \
                 tc.tile_pool(name="exps", bufs=3) as exp_pool, \
                 tc.tile_pool(name="tails", bufs=2) as tails, \
                 tc.tile_pool(name="norms", bufs=2) as norms, \
                 tc.tile_pool(name="fins", bufs=2) as fins:
                for qb in range(2):
                    # onrm[hg] accumulates the 4 normalized heads of chunk hg
                    onrm0 = norms.tile([128, 512], f32, tag="onrm0")
                    onrm1 = norms.tile([128, 512], f32, tag="onrm1")
                    onrm_tiles = [onrm0, onrm1]
                    for hp in range(4):          # head pairs
                        hg, sub = hp // 2, hp % 2
                        # concurrent tile_position row-groups must land in
                        # SEPARATE psum banks (same-bank pairs fault the PE)
                        po = o_pool.tile([D + 1, 2, 512], f32, tag="opo")
                        for kc in range(24):
                            ps = s_pool.tile([128, 2, 512], f32, tag="S")
                            for j in range(2):
                                pof = 64 * sub + 32 * j
                                nc.tensor.matmul(
                                    ps[:, j, :],
                                    lhsT=k_sb[pof:pof + 32, hg, kc * 128:(kc + 1) * 128],
                                    rhs=q_sb[pof:pof + 32, hg, qb * 512:(qb + 1) * 512],
                                    start=True, stop=True, tile_position=(pof, 0))
                            slot = (qb * 4 + hp) * 24 + kc
                            if _use_dve_exp(slot):
                                es_i = exp_pool.tile([128, 2, 512], i32, tag="exp")
                                nc.vector.tensor_scalar(
                                    out=es_i, in0=ps, scalar1=_SCHR_A, scalar2=_SCHR_B,
                                    op0=Alu.mult, op1=Alu.add)
                                es = es_i.bitcast(f32)
                            else:
                                es = exp_pool.tile([128, 2, 512], f32, tag="exp")
                                nc.scalar.activation(es, ps, Act.Exp, scale=SCALE)
                            for j in range(2):
                                nc.tensor.matmul(
                                    po[:, j, :],
                                    lhsT=vT_aug[:, kc, hp * 2 + j, :],
                                    rhs=es[:, j, :],
                                    start=(kc == 0), stop=(kc == 23))
                        # tail: numerators + softmax denominators
                        raw = tails.tile([D + 1, 2, 512], f32, tag="raw")
                        nc.vector.tensor_copy(raw, po)
                        # denominators live on ONE partition row; iterative
                        # reciprocal is 8cyc/elem/lane, so spread the 1024
                        # values over 32 partitions via DMA, recip, pack back
                        dp = tails.tile([32, 32], f32, tag="dp")
                        nc.sync.dma_start(
                            out=dp, in_=raw[D:D + 1, :, :].rearrange("p a q -> p (a q)"))
                        rp = tails.tile([32, 32], f32, tag="rp")
                        nc.vector.reciprocal(rp, dp)
                        rec = tails.tile([1, 2, 512], f32, tag="rec")
                        nc.sync.dma_start(
                            out=rec.rearrange("p a q -> p (a q)"), in_=rp)
                        oin = tails.tile([128, 512], f32, tag="oin")
                        rbc = tails.tile([128, 512], f32, tag="rbc")
                        for j in range(2):
                            pof = 64 * sub + 32 * j
                            nc.sync.dma_start(out=oin[pof:pof + 32, :],
                                              in_=raw[0:D, j, :])
                            nc.sync.dma_start(out=rbc[pof:pof + 32, :],
                                              in_=bcast_sbuf_row(rec[0:1, j, :], 32))
                        nc.vector.tensor_mul(
                            onrm_tiles[hg][64 * sub:64 * sub + 64, :],
                            oin[64 * sub:64 * sub + 64, :],
                            rbc[64 * sub:64 * sub + 64, :])
                    # o-projection + residual + bias + LayerNorm per 128 queries
                    for qc2 in range(4):
                        qoff = qb * 512 + qc2 * 128
                        pso = s_pool.tile([128, C], f32, tag="S")
                        for hgc in range(2):
                            nc.tensor.matmul(
                                pso, lhsT=onrm_tiles[hgc][:, qc2 * 128:(qc2 + 1) * 128],
                                rhs=w_sb["owT"][:, hgc, :],
                                start=(hgc == 0), stop=False)
                        for cc in range(2):
                            nc.tensor.matmul(
                                pso, lhsT=x_sb[:, cc, qoff:qoff + 128],
                                rhs=w_sb["ident"][:, cc, :], start=False, stop=False)
                        nc.tensor.matmul(pso, lhsT=ones_row[0:1, 0:128], rhs=ob_row[:],
                                         start=False, stop=True)
                        stats = fins.tile([128, 6], f32, tag="stats")
                        nc.vector.bn_stats(stats, pso)
                        mv = fins.tile([128, 2], f32, tag="mv")
                        nc.vector.bn_aggr(mv, stats)
                        # rstd = exp(-0.5*ln(var+eps)): stays in the same ACT
                        # table set as the softmax exp (no table reload)
                        lnv = fins.tile([128, 1], f32, tag="lnv")
                        nc.scalar.activation(lnv, mv[:, 1:2], Act.Ln, bias=eps_col[:, 0:1])
                        rstd = fins.tile([128, 1], f32, tag="rstd")
                        nc.scalar.activation(rstd, lnv, Act.Exp, scale=-0.5)
                        t1 = fins.tile([128, C], f32, tag="t1")
                        nc.vector.tensor_scalar(
                            out=t1, in0=pso, scalar1=mv[:, 0:1], scalar2=rstd,
                            op0=Alu.subtract, op1=Alu.mult)
                        t2 = fins.tile([128, C], f32, tag="t2")
                        nc.vector.tensor_mul(t2, t1, lnw_bc)
                        t3 = fins.tile([128, C], f32, tag="t3")
                        nc.vector.tensor_add(t3, t2, lnb_bc)
                        nc.sync.dma_start(out=y_d[qoff:qoff + 128, :], in_=t3)
    return nc


_CACHE = {}


def _get_program():
    if "nc" not in _CACHE:
        _apply_walrus_wait_patch()
        _CACHE["nc"] = build_program()
    return _CACHE["nc"]


def _make_in_maps(inputs):
    s3 = np.ascontiguousarray(np.asarray(inputs["s3"], dtype=np.float32))
    s4 = np.ascontiguousarray(np.asarray(inputs["s4"], dtype=np.float32))
    s5 = np.ascontiguousarray(np.asarray(inputs["s5"], dtype=np.float32))
    B = s3.shape[0]
    wts = {}
    for nm in ("qw", "kw", "vw", "ow"):
        wts[nm + "T"] = np.ascontiguousarray(np.asarray(inputs[nm], dtype=np.float32).T)
    ident = np.eye(C, dtype=np.float32)
    rows = {}
    for nm in ("qb", "kb", "vb", "ob", "ln_w", "ln_b"):
        rows[nm] = np.ascontiguousarray(
            np.asarray(inputs[nm], dtype=np.float32).reshape(1, C))
    in_maps = []
    for core in range(N_CORES):
        b, qc = core // 4, core % 4
        x = np.ascontiguousarray(
            s3[b].reshape(C, -1)[:, qc * NQ_CORE:(qc + 1) * NQ_CORE])
        kv = np.ascontiguousarray(np.concatenate(
            [s4[b].reshape(C, -1), s5[b].reshape(C, -1)], axis=1))
        in_maps.append({
            "x": x, "kv": kv,
            "qwT": wts["qwT"], "kwT": wts["kwT"], "vwT": wts["vwT"],
            "owT": wts["owT"], "ident": ident,
            "qb2": rows["qb"], "kb2": rows["kb"], "vb2": rows["vb"],
            "ob2": rows["ob"], "lnw2": rows["ln_w"], "lnb2": rows["ln_b"],
        })
    return in_maps


def _assemble(results, like):
    B, _, H, W = 2, C, 64, 64
    out = np.empty((B, C, H * W), dtype=np.float32)
    for core in range(N_CORES):
        b, qc = core // 4, core % 4
        out[b, :, qc * NQ_CORE:(qc + 1) * NQ_CORE] = results[core]["y"].T
    return out.reshape(B, C, H, W)


def kernel(**inputs):
    from concourse import bass2jax
    nc = _get_program()
    in_maps = _make_in_maps(inputs)
    results = bass2jax.run_bass_via_pjrt(nc, in_maps, n_cores=N_CORES)
    return _assemble(results, inputs["s3"])



# revision 2
# speedup vs baseline: 1.1366x; 1.1366x over previous
"""Content-guided attention kernel for Trainium2, 8 NeuronCores SPMD.

Sharding: 8 cores = (batch b in {0,1}) x (query-chunk qc in {0..3}).
Each core computes 1024 query positions of batch b end-to-end:
q/k/vT projections, 8-head attention over all 3072 keys, o-projection,
residual and LayerNorm.  No collectives needed; host splits/concats.

Per-core layout highlights:
 - all matmul operands are bf16 (fp32 PSUM accumulation): fp32 matmuls
   run as 2 HW passes each, bf16 runs single-pass and enables FWL for
   the per-k-chunk score weight loads, roughly halving PE busy time
 - scores computed transposed S^T[kpos, qpos] so softmax sum folds into the
   attn@V matmul via a ones-column appended to V^T (no partition reductions)
 - head_dim=32 scores matmuls are packed 4-at-a-time into the PE's 32-row
   groups via tile_position (4x concurrency at K=32)
 - exp split between ScalarE (exact table exp -> bf16 out) and VectorE
   (Schraudolph bit-trick exp in int16 -> bitcast bf16, ~3% elementwise,
   cancels in softmax normalization)
 - LayerNorm rstd computed as exp(-0.5*ln(var+eps)) to stay inside the
   single natural_log_exp ACT table set (no table switch thrash)
"""

import numpy as np
import ml_dtypes

BF16 = ml_dtypes.bfloat16

C = 256
NH = 8
D = 32
NQ_CORE = 1024
NK = 3072
N_CORES = 8
SCALE = float(D) ** -0.5

# Schraudolph exp constants for int16/bfloat16 bits (validated offline:
# 3.3% max elem rel err on the observed score range; cancels in softmax).
_SCHR_A16 = float(np.float32(SCALE * (1 << 7) / np.log(2.0)))
_SCHR_B16 = float(np.float32(127.0 * (1 << 7) - 365000.0 / 65536.0))

# every 3rd exp slot goes to the vector engine to offload the ACT bottleneck
def _use_dve_exp(slot: int) -> bool:
    return slot % 3 == 2


def _apply_walrus_wait_patch():
    """This walrus build accepts only ONE sync-wait per instruction; split
    extra waits onto single-wait NoOps inserted before the instruction
    (same engine, same block => per-engine program order preserved)."""
    import orjson
    import concourse.bass_utils as bass_utils
    import concourse.bass2jax as bass2jax

    if getattr(bass_utils, "_ant_wait_split_patch", False):
        return
    bass_utils._ant_wait_split_patch = True
    counter = [0]

    def _split_waits(bir_bytes: bytes) -> bytes:
        d = orjson.loads(bir_bytes)
        changed = False

        def process_blocks(blocks):
            nonlocal changed
            for b in blocks:
                insts = b.get("instructions")
                if insts:
                    new = []
                    for ins in insts:
                        si = ins.get("sync_info")
                        waits = si.get("on_wait") if si else None
                        if waits and len(waits) > 1:
                            changed = True
                            for w in waits[:-1]:
                                counter[0] += 1
                                new.append({
                                    "debug": ins.get("debug", 0),
                                    "engine": ins["engine"],
                                    "ins": [],
                                    "outs": [],
                                    "name": f"antwsplit-{counter[0]}",
                                    "opcode": "NoOp",
                                    "sync_info": {"on_wait": [w], "on_update": []},
                                })
                            si["on_wait"] = [waits[-1]]
                        new.append(ins)
                    b["instructions"] = new
                if b.get("blocks"):
                    process_blocks(b["blocks"])

        for f in d.get("functions", []):
            process_blocks(f.get("blocks", []))
        return orjson.dumps(d) if changed else bir_bytes

    orig = bass_utils.compile_bir_kernel

    def compile_bir_kernel(bir, tmpdir, neff_name="file.neff", **kw):
        if isinstance(bir, (bytes, bytearray)):
            bir = _split_waits(bytes(bir))
        elif isinstance(bir, str):
            bir = _split_waits(bir.encode()).decode()
        return orig(bir, tmpdir, neff_name=neff_name, **kw)

    bass_utils.compile_bir_kernel = compile_bir_kernel
    bass2jax.compile_bir_kernel = compile_bir_kernel


def build_program():
    import concourse.bass as bass
    import concourse.tile as tile
    from concourse import mybir

    f32 = mybir.dt.float32
    bf16 = mybir.dt.bfloat16
    i16 = mybir.dt.int16
    Alu = mybir.AluOpType
    Act = mybir.ActivationFunctionType

    nc = bass.Bass()

    x_d = nc.dram_tensor("x", [C, NQ_CORE], bf16, kind="ExternalInput")
    kv_d = nc.dram_tensor("kv", [C, NK], bf16, kind="ExternalInput")
    qwT_d = nc.dram_tensor("qwT", [C, C], bf16, kind="ExternalInput")
    kwT_d = nc.dram_tensor("kwT", [C, C], bf16, kind="ExternalInput")
    vwT_d = nc.dram_tensor("vwT", [C, C], bf16, kind="ExternalInput")
    owT_d = nc.dram_tensor("owT", [C, C], bf16, kind="ExternalInput")
    ident_d = nc.dram_tensor("ident", [C, C], bf16, kind="ExternalInput")
    qb_d = nc.dram_tensor("qb2", [1, C], bf16, kind="ExternalInput")
    kb_d = nc.dram_tensor("kb2", [1, C], bf16, kind="ExternalInput")
    vb_d = nc.dram_tensor("vb2", [1, C], bf16, kind="ExternalInput")
    ob_d = nc.dram_tensor("ob2", [1, C], bf16, kind="ExternalInput")
    lnw_d = nc.dram_tensor("lnw2", [1, C], f32, kind="ExternalInput")
    lnb_d = nc.dram_tensor("lnb2", [1, C], f32, kind="ExternalInput")
    y_d = nc.dram_tensor("y", [NQ_CORE, C], f32, kind="ExternalOutput")

    def bcast_part(ap, n):
        # partition-stride-0 view: replicate one partition row across n
        # (DRAM sources only; SBUF partition dims need nonzero step)
        return bass.AP(tensor=ap.tensor, offset=ap.offset,
                       ap=[[0, n]] + [list(a) for a in ap.ap[1:]])

    def bcast_sbuf_row(ap, n):
        # SBUF [1, F] row -> [n, F] DMA source: keep the 1-partition dim,
        # replicate via a step-0 free dim (legal for DMA reads)
        return bass.AP(tensor=ap.tensor, offset=ap.offset,
                       ap=[list(ap.ap[0]), [0, n]] + [list(a) for a in ap.ap[1:]])

    from contextlib import ExitStack
    with tile.TileContext(nc) as tc, ExitStack() as ctx:
            consts = ctx.enter_context(tc.tile_pool(name="consts", bufs=1))
            data = ctx.enter_context(tc.tile_pool(name="data", bufs=1))
            acts = ctx.enter_context(tc.tile_pool(name="acts", bufs=1))
            # ---- constants ----
            w_sb = {}
            for nm, dt_ in (("qwT", qwT_d), ("kwT", kwT_d), ("vwT", vwT_d),
                            ("owT", owT_d), ("ident", ident_d)):
                t = consts.tile([128, 2, C], bf16, tag=f"w_{nm}")
                nc.sync.dma_start(out=t, in_=dt_.rearrange("(a p) c -> p a c", p=128))
                w_sb[nm] = t
            qb_row = consts.tile([1, C], bf16, tag="qb_row")
            kb_row = consts.tile([1, C], bf16, tag="kb_row")
            vb_row = consts.tile([1, C], bf16, tag="vb_row")
            ob_row = consts.tile([1, C], bf16, tag="ob_row")
            for t, dt_ in ((qb_row, qb_d), (kb_row, kb_d), (vb_row, vb_d), (ob_row, ob_d)):
                nc.sync.dma_start(out=t, in_=dt_[:])
            lnw_bc = consts.tile([128, C], f32, tag="lnw_bc")
            lnb_bc = consts.tile([128, C], f32, tag="lnb_bc")
            nc.sync.dma_start(out=lnw_bc, in_=bcast_part(lnw_d[:], 128))
            nc.sync.dma_start(out=lnb_bc, in_=bcast_part(lnb_d[:], 128))
            ones_row = consts.tile([1, 512], bf16, tag="ones_row")
            nc.vector.memset(ones_row, 1.0)
            eps_col = consts.tile([128, 1], f32, tag="eps_col")
            nc.vector.memset(eps_col, 1e-5)

            # ---- input activations ----
            x_sb = data.tile([128, 2, NQ_CORE], bf16, tag="x_sb")
            nc.sync.dma_start(out=x_sb, in_=x_d.rearrange("(a p) n -> p a n", p=128))
            kv_sb = data.tile([128, 2, NK], bf16, tag="kv_sb")
            nc.sync.dma_start(out=kv_sb, in_=kv_d.rearrange("(a p) n -> p a n", p=128))

            q_sb = acts.tile([128, 2, NQ_CORE], bf16, tag="q_sb")
            k_sb = acts.tile([128, 2, NK], bf16, tag="k_sb")
            vT_aug = acts.tile([128, 24, NH, D + 1], bf16, tag="vT_aug")
            nc.vector.memset(vT_aug[:, :, :, D:D + 1], 1.0)

            # ---- projections ----
            with tc.tile_pool(name="proj_ps", bufs=4, space="PSUM") as proj_ps:
                # q = qw @ x + qb   (chunks of output channels x 512 cols)
                for mc in range(2):
                    for nb in range(2):
                        ps = proj_ps.tile([128, 512], f32, tag="proj")
                        for kc2 in range(2):
                            nc.tensor.matmul(
                                ps, lhsT=w_sb["qwT"][:, kc2, mc * 128:(mc + 1) * 128],
                                rhs=x_sb[:, kc2, nb * 512:(nb + 1) * 512],
                                start=(kc2 == 0), stop=False)
                        nc.tensor.matmul(
                            ps, lhsT=qb_row[:, mc * 128:(mc + 1) * 128],
                            rhs=ones_row[:, 0:512], start=False, stop=True)
                        nc.vector.tensor_copy(q_sb[:, mc, nb * 512:(nb + 1) * 512], ps)
                # k = kw @ kv + kb
                for mc in range(2):
                    for nb in range(6):
                        ps = proj_ps.tile([128, 512], f32, tag="proj")
                        for kc2 in range(2):
                            nc.tensor.matmul(
                                ps, lhsT=w_sb["kwT"][:, kc2, mc * 128:(mc + 1) * 128],
                                rhs=kv_sb[:, kc2, nb * 512:(nb + 1) * 512],
                                start=(kc2 == 0), stop=False)
                        nc.tensor.matmul(
                            ps, lhsT=kb_row[:, mc * 128:(mc + 1) * 128],
                            rhs=ones_row[:, 0:512], start=False, stop=True)
                        nc.vector.tensor_copy(k_sb[:, mc, nb * 512:(nb + 1) * 512], ps)
                # vT[n, c] = (kv^T @ vw^T)[n, c] + vb[c], written per-head with
                # a ones column appended (softmax denominator trick)
                for nn in range(24):
                    ps = proj_ps.tile([128, C], f32, tag="proj")
                    for kc2 in range(2):
                        nc.tensor.matmul(
                            ps, lhsT=kv_sb[:, kc2, nn * 128:(nn + 1) * 128],
                            rhs=w_sb["vwT"][:, kc2, :], start=(kc2 == 0), stop=False)
                    nc.tensor.matmul(ps, lhsT=ones_row[0:1, 0:128], rhs=vb_row[:],
                                     start=False, stop=True)
                    nc.vector.tensor_copy(
                        vT_aug[:, nn, :, 0:D],
                        ps.rearrange("p (h e) -> p h e", h=NH))

            # ---- attention + o-proj + LN ----
            with tc.tile_pool(name="s_ps", bufs=3, space="PSUM") as s_pool, \
                 tc.tile_pool(name="o_ps", bufs=1, space="PSUM") as o_pool, \
                 tc.tile_pool(name="exps", bufs=3) as exp_pool, \
                 tc.tile_pool(name="tails", bufs=2) as tails, \
                 tc.tile_pool(name="norms", bufs=2) as norms, \
                 tc.tile_pool(name="fins", bufs=2) as fins:
                for qb in range(2):
                    # onrm[hg] accumulates the 4 normalized heads of chunk hg
                    onrm0 = norms.tile([128, 512], bf16, tag="onrm0")
                    onrm1 = norms.tile([128, 512], bf16, tag="onrm1")
                    onrm_tiles = [onrm0, onrm1]
                    for hp in range(4):          # head pairs
                        hg, sub = hp // 2, hp % 2
                        # concurrent tile_position row-groups must land in
                        # SEPARATE psum banks (same-bank pairs fault the PE)
                        po = o_pool.tile([D + 1, 2, 512], f32, tag="opo")
                        for kc in range(24):
                            ps = s_pool.tile([128, 2, 512], f32, tag="S")
                            for j in range(2):
                                pof = 64 * sub + 32 * j
                                nc.tensor.matmul(
                                    ps[:, j, :],
                                    lhsT=k_sb[pof:pof + 32, hg, kc * 128:(kc + 1) * 128],
                                    rhs=q_sb[pof:pof + 32, hg, qb * 512:(qb + 1) * 512],
                                    start=True, stop=True, tile_position=(pof, 0))
                            slot = (qb * 4 + hp) * 24 + kc
                            es = exp_pool.tile([128, 2, 512], bf16, tag="exp")
                            if _use_dve_exp(slot):
                                es_i = es.bitcast(i16)
                                nc.vector.tensor_scalar(
                                    out=es_i, in0=ps, scalar1=_SCHR_A16, scalar2=_SCHR_B16,
                                    op0=Alu.mult, op1=Alu.add)
                            else:
                                nc.scalar.activation(es, ps, Act.Exp, scale=SCALE)
                            for j in range(2):
                                nc.tensor.matmul(
                                    po[:, j, :],
                                    lhsT=vT_aug[:, kc, hp * 2 + j, :],
                                    rhs=es[:, j, :],
                                    start=(kc == 0), stop=(kc == 23))
                        # tail: numerators + softmax denominators
                        raw = tails.tile([D + 1, 2, 512], f32, tag="raw")
                        nc.vector.tensor_copy(raw, po)
                        # denominators live on ONE partition row; iterative
                        # reciprocal is 8cyc/elem/lane, so spread the 1024
                        # values over 32 partitions via DMA, recip, pack back
                        dp = tails.tile([32, 32], f32, tag="dp")
                        nc.sync.dma_start(
                            out=dp, in_=raw[D:D + 1, :, :].rearrange("p a q -> p (a q)"))
                        rp = tails.tile([32, 32], f32, tag="rp")
                        nc.vector.reciprocal(rp, dp)
                        rec = tails.tile([1, 2, 512], f32, tag="rec")
                        nc.sync.dma_start(
                            out=rec.rearrange("p a q -> p (a q)"), in_=rp)
                        oin = tails.tile([128, 512], f32, tag="oin")
                        rbc = tails.tile([128, 512], f32, tag="rbc")
                        for j in range(2):
                            pof = 64 * sub + 32 * j
                            nc.sync.dma_start(out=oin[pof:pof + 32, :],
                                              in_=raw[0:D, j, :])
                            nc.sync.dma_start(out=rbc[pof:pof + 32, :],
                                              in_=bcast_sbuf_row(rec[0:1, j, :], 32))
                        nc.vector.tensor_mul(
                            onrm_tiles[hg][64 * sub:64 * sub + 64, :],
                            oin[64 * sub:64 * sub + 64, :],
                            rbc[64 * sub:64 * sub + 64, :])
                    # o-projection + residual + bias + LayerNorm per 128 queries
                    for qc2 in range(4):
                        qoff = qb * 512 + qc2 * 128
                        pso = s_pool.tile([128, C], f32, tag="S")
                        for hgc in range(2):
                            nc.tensor.matmul(
                                pso, lhsT=onrm_tiles[hgc][:, qc2 * 128:(qc2 + 1) * 128],
                                rhs=w_sb["owT"][:, hgc, :],
                                start=(hgc == 0), stop=False)
                        for cc in range(2):
                            nc.tensor.matmul(
                                pso, lhsT=x_sb[:, cc, qoff:qoff + 128],
                                rhs=w_sb["ident"][:, cc, :], start=False, stop=False)
                        nc.tensor.matmul(pso, lhsT=ones_row[0:1, 0:128], rhs=ob_row[:],
                                         start=False, stop=True)
                        stats = fins.tile([128, 6], f32, tag="stats")
                        nc.vector.bn_stats(stats, pso)
                        mv = fins.tile([128, 2], f32, tag="mv")
                        nc.vector.bn_aggr(mv, stats)
                        # rstd = exp(-0.5*ln(var+eps)): stays in the same ACT
                        # table set as the softmax exp (no table reload)
                        lnv = fins.tile([128, 1], f32, tag="lnv")
                        nc.scalar.activation(lnv, mv[:, 1:2], Act.Ln, bias=eps_col[:, 0:1])
                        rstd = fins.tile([128, 1], f32, tag="rstd")
                        nc.scalar.activation(rstd, lnv, Act.Exp, scale=-0.5)
                        t1 = fins.tile([128, C], f32, tag="t1")
                        nc.vector.tensor_scalar(
                            out=t1, in0=pso, scalar1=mv[:, 0:1], scalar2=rstd,
                            op0=Alu.subtract, op1=Alu.mult)
                        t2 = fins.tile([128, C], f32, tag="t2")
                        nc.vector.tensor_mul(t2, t1, lnw_bc)
                        t3 = fins.tile([128, C], f32, tag="t3")
                        nc.vector.tensor_add(t3, t2, lnb_bc)
                        nc.sync.dma_start(out=y_d[qoff:qoff + 128, :], in_=t3)
    return nc


_CACHE = {}


def _get_program():
    if "nc" not in _CACHE:
        _apply_walrus_wait_patch()
        _CACHE["nc"] = build_program()
    return _CACHE["nc"]


def _make_in_maps(inputs):
    s3 = np.ascontiguousarray(np.asarray(inputs["s3"], dtype=np.float32))
    s4 = np.ascontiguousarray(np.asarray(inputs["s4"], dtype=np.float32))
    s5 = np.ascontiguousarray(np.asarray(inputs["s5"], dtype=np.float32))
    B = s3.shape[0]
    wts = {}
    for nm in ("qw", "kw", "vw", "ow"):
        wts[nm + "T"] = np.ascontiguousarray(
            np.asarray(inputs[nm], dtype=np.float32).T.astype(BF16))
    ident = np.eye(C, dtype=BF16)
    rows = {}
    for nm in ("qb", "kb", "vb", "ob"):
        rows[nm] = np.ascontiguousarray(
            np.asarray(inputs[nm], dtype=np.float32).reshape(1, C).astype(BF16))
    for nm in ("ln_w", "ln_b"):
        rows[nm] = np.ascontiguousarray(
            np.asarray(inputs[nm], dtype=np.float32).reshape(1, C))
    in_maps = []
    for core in range(N_CORES):
        b, qc = core // 4, core % 4
        x = np.ascontiguousarray(
            s3[b].reshape(C, -1)[:, qc * NQ_CORE:(qc + 1) * NQ_CORE].astype(BF16))
        kv = np.ascontiguousarray(np.concatenate(
            [s4[b].reshape(C, -1), s5[b].reshape(C, -1)], axis=1).astype(BF16))
        in_maps.append({
            "x": x, "kv": kv,
            "qwT": wts["qwT"], "kwT": wts["kwT"], "vwT": wts["vwT"],
            "owT": wts["owT"], "ident": ident,
            "qb2": rows["qb"], "kb2": rows["kb"], "vb2": rows["vb"],
            "ob2": rows["ob"], "lnw2": rows["ln_w"], "lnb2": rows["ln_b"],
        })
    return in_maps


def _assemble(results, like):
    B, _, H, W = 2, C, 64, 64
    out = np.empty((B, C, H * W), dtype=np.float32)
    for core in range(N_CORES):
        b, qc = core // 4, core % 4
        out[b, :, qc * NQ_CORE:(qc + 1) * NQ_CORE] = results[core]["y"].T
    return out.reshape(B, C, H, W)


def kernel(**inputs):
    from concourse import bass2jax
    nc = _get_program()
    in_maps = _make_in_maps(inputs)
    results = bass2jax.run_bass_via_pjrt(nc, in_maps, n_cores=N_CORES)
    return _assemble(results, inputs["s3"])


# revision 3
# speedup vs baseline: 249.6078x; 219.6107x over previous
"""Content-guided attention kernel for Trainium2, 8 NeuronCores SPMD.

Sharding: 8 cores = (batch b in {0,1}) x (query-chunk qc in {0..3}).
Each core computes 1024 query positions of batch b end-to-end:
q/k/vT projections, 8-head attention over all 3072 keys, o-projection,
residual and LayerNorm.  No collectives needed; host splits/concats.

Per-core layout highlights:
 - all matmul operands are bf16 (fp32 PSUM accumulation): fp32 matmuls
   run as 2 HW passes each, bf16 runs single-pass and enables FWL for
   the per-k-chunk score weight loads, roughly halving PE busy time
 - scores computed transposed S^T[kpos, qpos] so softmax sum folds into the
   attn@V matmul via a ones-column appended to V^T (no partition reductions)
 - head_dim=32 scores matmuls are packed 4-at-a-time into the PE's 32-row
   groups via tile_position (4x concurrency at K=32)
 - exp split between ScalarE (exact table exp -> bf16 out) and VectorE
   (Schraudolph bit-trick exp in int16 -> bitcast bf16, ~3% elementwise,
   cancels in softmax normalization)
 - LayerNorm rstd computed as exp(-0.5*ln(var+eps)) to stay inside the
   single natural_log_exp ACT table set (no table switch thrash)
"""

import numpy as np
import ml_dtypes

BF16 = ml_dtypes.bfloat16

C = 256
NH = 8
D = 32
NQ_CORE = 1024
NK = 3072
N_CORES = 8
SCALE = float(D) ** -0.5

# Schraudolph exp constants for int16/bfloat16 bits (validated offline:
# 3.3% max elem rel err on the observed score range; cancels in softmax).
_SCHR_A16 = float(np.float32(SCALE * (1 << 7) / np.log(2.0)))
_SCHR_B16 = float(np.float32(127.0 * (1 << 7) - 365000.0 / 65536.0))

# every 3rd exp slot goes to the vector engine to offload the ACT bottleneck
def _use_dve_exp(slot: int) -> bool:
    return slot % 3 == 2


def _apply_walrus_wait_patch():
    """This walrus build accepts only ONE sync-wait per instruction; split
    extra waits onto single-wait NoOps inserted before the instruction
    (same engine, same block => per-engine program order preserved)."""
    import orjson
    import concourse.bass_utils as bass_utils
    import concourse.bass2jax as bass2jax

    if getattr(bass_utils, "_ant_wait_split_patch", False):
        return
    bass_utils._ant_wait_split_patch = True
    counter = [0]

    def _split_waits(bir_bytes: bytes) -> bytes:
        d = orjson.loads(bir_bytes)
        changed = False

        def process_blocks(blocks):
            nonlocal changed
            for b in blocks:
                insts = b.get("instructions")
                if insts:
                    new = []
                    for ins in insts:
                        si = ins.get("sync_info")
                        waits = si.get("on_wait") if si else None
                        if waits and len(waits) > 1:
                            changed = True
                            for w in waits[:-1]:
                                counter[0] += 1
                                new.append({
                                    "debug": ins.get("debug", 0),
                                    "engine": ins["engine"],
                                    "ins": [],
                                    "outs": [],
                                    "name": f"antwsplit-{counter[0]}",
                                    "opcode": "NoOp",
                                    "sync_info": {"on_wait": [w], "on_update": []},
                                })
                            si["on_wait"] = [waits[-1]]
                        new.append(ins)
                    b["instructions"] = new
                if b.get("blocks"):
                    process_blocks(b["blocks"])

        for f in d.get("functions", []):
            process_blocks(f.get("blocks", []))
        return orjson.dumps(d) if changed else bir_bytes

    orig = bass_utils.compile_bir_kernel

    def compile_bir_kernel(bir, tmpdir, neff_name="file.neff", **kw):
        if isinstance(bir, (bytes, bytearray)):
            bir = _split_waits(bytes(bir))
        elif isinstance(bir, str):
            bir = _split_waits(bir.encode()).decode()
        return orig(bir, tmpdir, neff_name=neff_name, **kw)

    bass_utils.compile_bir_kernel = compile_bir_kernel
    bass2jax.compile_bir_kernel = compile_bir_kernel


def build_program():
    import concourse.bass as bass
    import concourse.tile as tile
    from concourse import mybir

    f32 = mybir.dt.float32
    bf16 = mybir.dt.bfloat16
    i16 = mybir.dt.int16
    Alu = mybir.AluOpType
    Act = mybir.ActivationFunctionType

    nc = bass.Bass()

    x_d = nc.dram_tensor("x", [C, NQ_CORE], bf16, kind="ExternalInput")
    kv_d = nc.dram_tensor("kv", [C, NK], bf16, kind="ExternalInput")
    qwT_d = nc.dram_tensor("qwT", [C, C], bf16, kind="ExternalInput")
    kwT_d = nc.dram_tensor("kwT", [C, C], bf16, kind="ExternalInput")
    vwT_d = nc.dram_tensor("vwT", [C, C], bf16, kind="ExternalInput")
    owT_d = nc.dram_tensor("owT", [C, C], bf16, kind="ExternalInput")
    ident_d = nc.dram_tensor("ident", [C, C], bf16, kind="ExternalInput")
    qb_d = nc.dram_tensor("qb2", [1, C], bf16, kind="ExternalInput")
    kb_d = nc.dram_tensor("kb2", [1, C], bf16, kind="ExternalInput")
    vb_d = nc.dram_tensor("vb2", [1, C], bf16, kind="ExternalInput")
    ob_d = nc.dram_tensor("ob2", [1, C], bf16, kind="ExternalInput")
    lnw_d = nc.dram_tensor("lnw2", [1, C], f32, kind="ExternalInput")
    lnb_d = nc.dram_tensor("lnb2", [1, C], f32, kind="ExternalInput")
    y_d = nc.dram_tensor("y", [NQ_CORE, C], f32, kind="ExternalOutput")

    def bcast_part(ap, n):
        # partition-stride-0 view: replicate one partition row across n
        # (DRAM sources only; SBUF partition dims need nonzero step)
        return bass.AP(tensor=ap.tensor, offset=ap.offset,
                       ap=[[0, n]] + [list(a) for a in ap.ap[1:]])

    def bcast_sbuf_row(ap, n):
        # SBUF [1, F] row -> [n, F] DMA source: keep the 1-partition dim,
        # replicate via a step-0 free dim (legal for DMA reads)
        return bass.AP(tensor=ap.tensor, offset=ap.offset,
                       ap=[list(ap.ap[0]), [0, n]] + [list(a) for a in ap.ap[1:]])

    from contextlib import ExitStack
    with tile.TileContext(nc) as tc, ExitStack() as ctx:
            consts = ctx.enter_context(tc.tile_pool(name="consts", bufs=1))
            data = ctx.enter_context(tc.tile_pool(name="data", bufs=1))
            acts = ctx.enter_context(tc.tile_pool(name="acts", bufs=1))
            # ---- constants ----
            w_sb = {}
            for nm, dt_ in (("qwT", qwT_d), ("kwT", kwT_d), ("vwT", vwT_d),
                            ("owT", owT_d), ("ident", ident_d)):
                t = consts.tile([128, 2, C], bf16, tag=f"w_{nm}")
                nc.sync.dma_start(out=t, in_=dt_.rearrange("(a p) c -> p a c", p=128))
                w_sb[nm] = t
            qb_row = consts.tile([1, C], bf16, tag="qb_row")
            kb_row = consts.tile([1, C], bf16, tag="kb_row")
            vb_row = consts.tile([1, C], bf16, tag="vb_row")
            ob_row = consts.tile([1, C], bf16, tag="ob_row")
            for t, dt_ in ((qb_row, qb_d), (kb_row, kb_d), (vb_row, vb_d), (ob_row, ob_d)):
                nc.sync.dma_start(out=t, in_=dt_[:])
            lnw_bc = consts.tile([128, C], f32, tag="lnw_bc")
            lnb_bc = consts.tile([128, C], f32, tag="lnb_bc")
            nc.sync.dma_start(out=lnw_bc, in_=bcast_part(lnw_d[:], 128))
            nc.sync.dma_start(out=lnb_bc, in_=bcast_part(lnb_d[:], 128))
            ones_row = consts.tile([1, 512], bf16, tag="ones_row")
            nc.vector.memset(ones_row, 1.0)
            eps_col = consts.tile([128, 1], f32, tag="eps_col")
            nc.vector.memset(eps_col, 1e-5)

            # ---- input activations ----
            x_sb = data.tile([128, 2, NQ_CORE], bf16, tag="x_sb")
            nc.sync.dma_start(out=x_sb, in_=x_d.rearrange("(a p) n -> p a n", p=128))
            kv_sb = data.tile([128, 2, NK], bf16, tag="kv_sb")
            nc.sync.dma_start(out=kv_sb, in_=kv_d.rearrange("(a p) n -> p a n", p=128))

            q_sb = acts.tile([128, 2, NQ_CORE], bf16, tag="q_sb")
            k_sb = acts.tile([128, 2, NK], bf16, tag="k_sb")
            vT_aug = acts.tile([128, 24, NH, D + 1], bf16, tag="vT_aug")
            nc.vector.memset(vT_aug[:, :, :, D:D + 1], 1.0)

            # ---- projections ----
            with tc.tile_pool(name="proj_ps", bufs=4, space="PSUM") as proj_ps:
                # q = qw @ x + qb   (chunks of output channels x 512 cols)
                for mc in range(2):
                    for nb in range(2):
                        ps = proj_ps.tile([128, 512], f32, tag="proj")
                        for kc2 in range(2):
                            nc.tensor.matmul(
                                ps, lhsT=w_sb["qwT"][:, kc2, mc * 128:(mc + 1) * 128],
                                rhs=x_sb[:, kc2, nb * 512:(nb + 1) * 512],
                                start=(kc2 == 0), stop=False)
                        nc.tensor.matmul(
                            ps, lhsT=qb_row[:, mc * 128:(mc + 1) * 128],
                            rhs=ones_row[:, 0:512], start=False, stop=True)
                        nc.vector.tensor_copy(q_sb[:, mc, nb * 512:(nb + 1) * 512], ps)
                # k = kw @ kv + kb
                for mc in range(2):
                    for nb in range(6):
                        ps = proj_ps.tile([128, 512], f32, tag="proj")
                        for kc2 in range(2):
                            nc.tensor.matmul(
                                ps, lhsT=w_sb["kwT"][:, kc2, mc * 128:(mc + 1) * 128],
                                rhs=kv_sb[:, kc2, nb * 512:(nb + 1) * 512],
                                start=(kc2 == 0), stop=False)
                        nc.tensor.matmul(
                            ps, lhsT=kb_row[:, mc * 128:(mc + 1) * 128],
                            rhs=ones_row[:, 0:512], start=False, stop=True)
                        nc.vector.tensor_copy(k_sb[:, mc, nb * 512:(nb + 1) * 512], ps)
                # vT[n, c] = (kv^T @ vw^T)[n, c] + vb[c], written per-head with
                # a ones column appended (softmax denominator trick)
                for nn in range(24):
                    ps = proj_ps.tile([128, C], f32, tag="proj")
                    for kc2 in range(2):
                        nc.tensor.matmul(
                            ps, lhsT=kv_sb[:, kc2, nn * 128:(nn + 1) * 128],
                            rhs=w_sb["vwT"][:, kc2, :], start=(kc2 == 0), stop=False)
                    nc.tensor.matmul(ps, lhsT=ones_row[0:1, 0:128], rhs=vb_row[:],
                                     start=False, stop=True)
                    nc.vector.tensor_copy(
                        vT_aug[:, nn, :, 0:D],
                        ps.rearrange("p (h e) -> p h e", h=NH))

            # ---- attention + o-proj + LN ----
            with tc.tile_pool(name="s_ps", bufs=3, space="PSUM") as s_pool, \
                 tc.tile_pool(name="o_ps", bufs=1, space="PSUM") as o_pool, \
                 tc.tile_pool(name="exps", bufs=3) as exp_pool, \
                 tc.tile_pool(name="tails", bufs=2) as tails, \
                 tc.tile_pool(name="norms", bufs=2) as norms, \
                 tc.tile_pool(name="fins", bufs=2) as fins:
                for qb in range(2):
                    # onrm[hg] accumulates the 4 normalized heads of chunk hg
                    onrm0 = norms.tile([128, 512], bf16, tag="onrm0")
                    onrm1 = norms.tile([128, 512], bf16, tag="onrm1")
                    onrm_tiles = [onrm0, onrm1]
                    for hp in range(4):          # head pairs
                        hg, sub = hp // 2, hp % 2
                        # the two heads of the pair accumulate CONCURRENTLY:
                        # col-groups 0-1 (partitions 0-32) and 2-3 (64-96)
                        po = o_pool.tile([128, 512], f32, tag="opo")
                        for kc in range(24):
                            ps = s_pool.tile([128, 2, 512], f32, tag="S")
                            for j in range(2):
                                pof = 64 * sub + 32 * j
                                nc.tensor.matmul(
                                    ps[:, j, :],
                                    lhsT=k_sb[pof:pof + 32, hg, kc * 128:(kc + 1) * 128],
                                    rhs=q_sb[pof:pof + 32, hg, qb * 512:(qb + 1) * 512],
                                    start=True, stop=True, tile_position=(pof, 0))
                            slot = (qb * 4 + hp) * 24 + kc
                            es = exp_pool.tile([128, 2, 512], bf16, tag="exp")
                            if _use_dve_exp(slot):
                                es_i = es.bitcast(i16)
                                nc.vector.tensor_scalar(
                                    out=es_i, in0=ps, scalar1=_SCHR_A16, scalar2=_SCHR_B16,
                                    op0=Alu.mult, op1=Alu.add)
                            else:
                                nc.scalar.activation(es, ps, Act.Exp, scale=SCALE)
                            for j in range(2):
                                nc.tensor.matmul(
                                    po[64 * j:64 * j + D + 1, :],
                                    lhsT=vT_aug[:, kc, hp * 2 + j, :],
                                    rhs=es[:, j, :],
                                    start=(kc == 0), stop=(kc == 23),
                                    tile_position=(0, 64 * j))
                        # tail: numerators + softmax denominators.  One copy
                        # moves both heads (lanes are parallel; cost = 512 cols)
                        raw = tails.tile([128, 512], f32, tag="raw")
                        nc.vector.tensor_copy(raw, po)
                        # denominators live on ONE partition row per head;
                        # iterative reciprocal is 8cyc/elem/lane, so spread the
                        # 1024 values over 32 partitions via DMA, recip, pack back
                        dp = tails.tile([32, 32], f32, tag="dp")
                        for j in range(2):
                            nc.sync.dma_start(
                                out=dp[16 * j:16 * j + 16, :],
                                in_=raw[64 * j + D:64 * j + D + 1, :])
                        rp = tails.tile([32, 32], f32, tag="rp")
                        nc.vector.reciprocal(rp, dp)
                        rec = tails.tile([1, 2, 512], f32, tag="rec")
                        nc.sync.dma_start(
                            out=rec.rearrange("p a q -> p (a q)"), in_=rp)
                        oin = tails.tile([128, 512], f32, tag="oin")
                        rbc = tails.tile([128, 512], f32, tag="rbc")
                        for j in range(2):
                            pof = 64 * sub + 32 * j
                            nc.sync.dma_start(out=oin[pof:pof + 32, :],
                                              in_=raw[64 * j:64 * j + D, :])
                            nc.sync.dma_start(out=rbc[pof:pof + 32, :],
                                              in_=bcast_sbuf_row(rec[0:1, j, :], 32))
                        nc.vector.tensor_mul(
                            onrm_tiles[hg][64 * sub:64 * sub + 64, :],
                            oin[64 * sub:64 * sub + 64, :],
                            rbc[64 * sub:64 * sub + 64, :])
                    # o-projection + residual + bias + LayerNorm per 128 queries
                    for qc2 in range(4):
                        qoff = qb * 512 + qc2 * 128
                        pso = s_pool.tile([128, C], f32, tag="S")
                        for hgc in range(2):
                            nc.tensor.matmul(
                                pso, lhsT=onrm_tiles[hgc][:, qc2 * 128:(qc2 + 1) * 128],
                                rhs=w_sb["owT"][:, hgc, :],
                                start=(hgc == 0), stop=False)
                        for cc in range(2):
                            nc.tensor.matmul(
                                pso, lhsT=x_sb[:, cc, qoff:qoff + 128],
                                rhs=w_sb["ident"][:, cc, :], start=False, stop=False)
                        nc.tensor.matmul(pso, lhsT=ones_row[0:1, 0:128], rhs=ob_row[:],
                                         start=False, stop=True)
                        stats = fins.tile([128, 6], f32, tag="stats")
                        nc.vector.bn_stats(stats, pso)
                        mv = fins.tile([128, 2], f32, tag="mv")
                        nc.vector.bn_aggr(mv, stats)
                        # rstd = exp(-0.5*ln(var+eps)): stays in the same ACT
                        # table set as the softmax exp (no table reload)
                        lnv = fins.tile([128, 1], f32, tag="lnv")
                        nc.scalar.activation(lnv, mv[:, 1:2], Act.Ln, bias=eps_col[:, 0:1])
                        rstd = fins.tile([128, 1], f32, tag="rstd")
                        nc.scalar.activation(rstd, lnv, Act.Exp, scale=-0.5)
                        t1 = fins.tile([128, C], f32, tag="t1")
                        nc.vector.tensor_scalar(
                            out=t1, in0=pso, scalar1=mv[:, 0:1], scalar2=rstd,
                            op0=Alu.subtract, op1=Alu.mult)
                        t2 = fins.tile([128, C], f32, tag="t2")
                        nc.vector.tensor_mul(t2, t1, lnw_bc)
                        t3 = fins.tile([128, C], f32, tag="t3")
                        nc.vector.tensor_add(t3, t2, lnb_bc)
                        nc.sync.dma_start(out=y_d[qoff:qoff + 128, :], in_=t3)
    return nc


_CACHE = {}


def _get_program():
    if "nc" not in _CACHE:
        _apply_walrus_wait_patch()
        _CACHE["nc"] = build_program()
    return _CACHE["nc"]


def _make_in_maps(inputs):
    s3 = np.ascontiguousarray(np.asarray(inputs["s3"], dtype=np.float32))
    s4 = np.ascontiguousarray(np.asarray(inputs["s4"], dtype=np.float32))
    s5 = np.ascontiguousarray(np.asarray(inputs["s5"], dtype=np.float32))
    B = s3.shape[0]
    wts = {}
    for nm in ("qw", "kw", "vw", "ow"):
        wts[nm + "T"] = np.ascontiguousarray(
            np.asarray(inputs[nm], dtype=np.float32).T.astype(BF16))
    ident = np.eye(C, dtype=BF16)
    rows = {}
    for nm in ("qb", "kb", "vb", "ob"):
        rows[nm] = np.ascontiguousarray(
            np.asarray(inputs[nm], dtype=np.float32).reshape(1, C).astype(BF16))
    for nm in ("ln_w", "ln_b"):
        rows[nm] = np.ascontiguousarray(
            np.asarray(inputs[nm], dtype=np.float32).reshape(1, C))
    in_maps = []
    for core in range(N_CORES):
        b, qc = core // 4, core % 4
        x = np.ascontiguousarray(
            s3[b].reshape(C, -1)[:, qc * NQ_CORE:(qc + 1) * NQ_CORE].astype(BF16))
        kv = np.ascontiguousarray(np.concatenate(
            [s4[b].reshape(C, -1), s5[b].reshape(C, -1)], axis=1).astype(BF16))
        in_maps.append({
            "x": x, "kv": kv,
            "qwT": wts["qwT"], "kwT": wts["kwT"], "vwT": wts["vwT"],
            "owT": wts["owT"], "ident": ident,
            "qb2": rows["qb"], "kb2": rows["kb"], "vb2": rows["vb"],
            "ob2": rows["ob"], "lnw2": rows["ln_w"], "lnb2": rows["ln_b"],
        })
    return in_maps


def _assemble(results, like):
    B, _, H, W = 2, C, 64, 64
    out = np.empty((B, C, H * W), dtype=np.float32)
    for core in range(N_CORES):
        b, qc = core // 4, core % 4
        out[b, :, qc * NQ_CORE:(qc + 1) * NQ_CORE] = results[core]["y"].T
    return out.reshape(B, C, H, W)


def kernel(**inputs):
    from concourse import bass2jax
    nc = _get_program()
    in_maps = _make_in_maps(inputs)
    results = bass2jax.run_bass_via_pjrt(nc, in_maps, n_cores=N_CORES)
    return _assemble(results, inputs["s3"])


# revision 9
# speedup vs baseline: 261.1544x; 1.0463x over previous
"""Content-guided attention kernel for Trainium2, 8 NeuronCores SPMD.

Sharding: 8 cores = (batch b in {0,1}) x (query-chunk qc in {0..3}).
Each core computes 1024 query positions of batch b end-to-end:
q/k/vT projections, 8-head attention over all 3072 keys, o-projection,
residual and LayerNorm.  No collectives needed; host splits/concats.

Per-core layout highlights:
 - all matmul operands are bf16 (fp32 PSUM accumulation): fp32 matmuls
   run as 2 HW passes each, bf16 runs single-pass and enables FWL for
   the per-k-chunk score weight loads, roughly halving PE busy time
 - scores computed transposed S^T[kpos, qpos] so softmax sum folds into the
   attn@V matmul via a ones-column appended to V^T (no partition reductions)
 - head_dim=32 scores matmuls are packed 4-at-a-time into the PE's 32-row
   groups via tile_position (4x concurrency at K=32)
 - exp split between ScalarE (exact table exp -> bf16 out) and VectorE
   (Schraudolph bit-trick exp in int16 -> bitcast bf16, ~3% elementwise,
   cancels in softmax normalization)
 - LayerNorm rstd computed as exp(-0.5*ln(var+eps)) to stay inside the
   single natural_log_exp ACT table set (no table switch thrash)
"""

import numpy as np
import ml_dtypes

BF16 = ml_dtypes.bfloat16

C = 256
NH = 8
D = 32
NQ_CORE = 1024
NK = 3072
N_CORES = 8
SCALE = float(D) ** -0.5

# Schraudolph exp constants for int16/bfloat16 bits (validated offline:
# 3.3% max elem rel err on the observed score range; cancels in softmax).
_SCHR_A16 = float(np.float32(SCALE * (1 << 7) / np.log(2.0)))
_SCHR_B16 = float(np.float32(127.0 * (1 << 7) - 365000.0 / 65536.0))

# every 3rd exp slot goes to the vector engine to offload the ACT bottleneck
def _use_dve_exp(slot: int) -> bool:
    return slot % 3 == 2


def _apply_walrus_wait_patch():
    """This walrus build accepts only ONE sync-wait per instruction; split
    extra waits onto single-wait NoOps inserted before the instruction
    (same engine, same block => per-engine program order preserved)."""
    import orjson
    import concourse.bass_utils as bass_utils
    import concourse.bass2jax as bass2jax

    if getattr(bass_utils, "_ant_wait_split_patch", False):
        return
    bass_utils._ant_wait_split_patch = True
    counter = [0]

    def _split_waits(bir_bytes: bytes) -> bytes:
        d = orjson.loads(bir_bytes)
        changed = False

        def process_blocks(blocks):
            nonlocal changed
            for b in blocks:
                insts = b.get("instructions")
                if insts:
                    new = []
                    for ins in insts:
                        si = ins.get("sync_info")
                        waits = si.get("on_wait") if si else None
                        if waits and len(waits) > 1:
                            changed = True
                            for w in waits[:-1]:
                                counter[0] += 1
                                new.append({
                                    "debug": ins.get("debug", 0),
                                    "engine": ins["engine"],
                                    "ins": [],
                                    "outs": [],
                                    "name": f"antwsplit-{counter[0]}",
                                    "opcode": "NoOp",
                                    "sync_info": {"on_wait": [w], "on_update": []},
                                })
                            si["on_wait"] = [waits[-1]]
                        new.append(ins)
                    b["instructions"] = new
                if b.get("blocks"):
                    process_blocks(b["blocks"])

        for f in d.get("functions", []):
            process_blocks(f.get("blocks", []))
        return orjson.dumps(d) if changed else bir_bytes

    orig = bass_utils.compile_bir_kernel

    def compile_bir_kernel(bir, tmpdir, neff_name="file.neff", **kw):
        if isinstance(bir, (bytes, bytearray)):
            bir = _split_waits(bytes(bir))
        elif isinstance(bir, str):
            bir = _split_waits(bir.encode()).decode()
        return orig(bir, tmpdir, neff_name=neff_name, **kw)

    bass_utils.compile_bir_kernel = compile_bir_kernel
    bass2jax.compile_bir_kernel = compile_bir_kernel


def build_program():
    import concourse.bass as bass
    import concourse.tile as tile
    from concourse import mybir

    f32 = mybir.dt.float32
    bf16 = mybir.dt.bfloat16
    i16 = mybir.dt.int16
    Alu = mybir.AluOpType
    Act = mybir.ActivationFunctionType

    nc = bass.Bass()

    x_d = nc.dram_tensor("x", [C, NQ_CORE], bf16, kind="ExternalInput")
    kv_d = nc.dram_tensor("kv", [C, NK], bf16, kind="ExternalInput")
    qwT_d = nc.dram_tensor("qwT", [C, C], bf16, kind="ExternalInput")
    kwT_d = nc.dram_tensor("kwT", [C, C], bf16, kind="ExternalInput")
    vwT_d = nc.dram_tensor("vwT", [C, C], bf16, kind="ExternalInput")
    owT_d = nc.dram_tensor("owT", [C, C], bf16, kind="ExternalInput")
    ident_d = nc.dram_tensor("ident", [C, C], bf16, kind="ExternalInput")
    qb_d = nc.dram_tensor("qb2", [1, C], bf16, kind="ExternalInput")
    kb_d = nc.dram_tensor("kb2", [1, C], bf16, kind="ExternalInput")
    vb_d = nc.dram_tensor("vb2", [1, C], bf16, kind="ExternalInput")
    ob_d = nc.dram_tensor("ob2", [1, C], bf16, kind="ExternalInput")
    lnw_d = nc.dram_tensor("lnw2", [1, C], f32, kind="ExternalInput")
    lnb_d = nc.dram_tensor("lnb2", [1, C], f32, kind="ExternalInput")
    y_d = nc.dram_tensor("y", [NQ_CORE, C], f32, kind="ExternalOutput")

    def bcast_part(ap, n):
        # partition-stride-0 view: replicate one partition row across n
        # (DRAM sources only; SBUF partition dims need nonzero step)
        return bass.AP(tensor=ap.tensor, offset=ap.offset,
                       ap=[[0, n]] + [list(a) for a in ap.ap[1:]])

    def bcast_sbuf_row(ap, n):
        # SBUF [1, F] row -> [n, F] DMA source: keep the 1-partition dim,
        # replicate via a step-0 free dim (legal for DMA reads)
        return bass.AP(tensor=ap.tensor, offset=ap.offset,
                       ap=[list(ap.ap[0]), [0, n]] + [list(a) for a in ap.ap[1:]])

    from contextlib import ExitStack
    with tile.TileContext(nc) as tc, ExitStack() as ctx:
            consts = ctx.enter_context(tc.tile_pool(name="consts", bufs=1))
            data = ctx.enter_context(tc.tile_pool(name="data", bufs=1))
            acts = ctx.enter_context(tc.tile_pool(name="acts", bufs=1))
            # ---- constants ----
            w_sb = {}
            for nm, dt_ in (("qwT", qwT_d), ("kwT", kwT_d), ("vwT", vwT_d),
                            ("owT", owT_d), ("ident", ident_d)):
                t = consts.tile([128, 2, C], bf16, tag=f"w_{nm}")
                nc.sync.dma_start(out=t, in_=dt_.rearrange("(a p) c -> p a c", p=128))
                w_sb[nm] = t
            qb_row = consts.tile([1, C], bf16, tag="qb_row")
            kb_row = consts.tile([1, C], bf16, tag="kb_row")
            vb_row = consts.tile([1, C], bf16, tag="vb_row")
            ob_row = consts.tile([1, C], bf16, tag="ob_row")
            for t, dt_ in ((qb_row, qb_d), (kb_row, kb_d), (vb_row, vb_d), (ob_row, ob_d)):
                nc.sync.dma_start(out=t, in_=dt_[:])
            lnw_bc = consts.tile([128, C], f32, tag="lnw_bc")
            lnb_bc = consts.tile([128, C], f32, tag="lnb_bc")
            nc.sync.dma_start(out=lnw_bc, in_=bcast_part(lnw_d[:], 128))
            nc.sync.dma_start(out=lnb_bc, in_=bcast_part(lnb_d[:], 128))
            ones_row = consts.tile([1, 512], bf16, tag="ones_row")
            nc.vector.memset(ones_row, 1.0)
            eps_col = consts.tile([128, 1], f32, tag="eps_col")
            nc.vector.memset(eps_col, 1e-5)

            # ---- input activations ----
            # x first (q-proj can start on it immediately); kv in halves so
            # k/v-proj start before the full tensor lands
            x_sb = data.tile([128, 2, NQ_CORE], bf16, tag="x_sb")
            nc.sync.dma_start(out=x_sb, in_=x_d.rearrange("(a p) n -> p a n", p=128))
            kv_half = []
            for h in range(2):
                t = data.tile([128, 2, NK // 2], bf16, tag=f"kv_sb{h}")
                nc.sync.dma_start(
                    out=t,
                    in_=kv_d[:, h * (NK // 2):(h + 1) * (NK // 2)]
                        .rearrange("(a p) n -> p a n", p=128))
                kv_half.append(t)

            q_sb = acts.tile([128, 2, NQ_CORE], bf16, tag="q_sb")
            k_sb = acts.tile([128, 2, NK], bf16, tag="k_sb")
            vT_aug = acts.tile([128, 24, NH, D + 1], bf16, tag="vT_aug")
            nc.vector.memset(vT_aug[:, :, :, D:D + 1], 1.0)

            # ---- projections ----
            with tc.tile_pool(name="proj_ps", bufs=4, space="PSUM") as proj_ps:
                # q = qw @ x + qb   (chunks of output channels x 512 cols)
                for mc in range(2):
                    for nb in range(2):
                        ps = proj_ps.tile([128, 512], f32, tag="proj")
                        for kc2 in range(2):
                            nc.tensor.matmul(
                                ps, lhsT=w_sb["qwT"][:, kc2, mc * 128:(mc + 1) * 128],
                                rhs=x_sb[:, kc2, nb * 512:(nb + 1) * 512],
                                start=(kc2 == 0), stop=False)
                        nc.tensor.matmul(
                            ps, lhsT=qb_row[:, mc * 128:(mc + 1) * 128],
                            rhs=ones_row[:, 0:512], start=False, stop=True)
                        nc.vector.tensor_copy(q_sb[:, mc, nb * 512:(nb + 1) * 512], ps)
                # k = kw @ kv + kb
                for mc in range(2):
                    for nb in range(6):
                        half, nbh = nb // 3, nb % 3
                        ps = proj_ps.tile([128, 512], f32, tag="proj")
                        for kc2 in range(2):
                            nc.tensor.matmul(
                                ps, lhsT=w_sb["kwT"][:, kc2, mc * 128:(mc + 1) * 128],
                                rhs=kv_half[half][:, kc2, nbh * 512:(nbh + 1) * 512],
                                start=(kc2 == 0), stop=False)
                        nc.tensor.matmul(
                            ps, lhsT=kb_row[:, mc * 128:(mc + 1) * 128],
                            rhs=ones_row[:, 0:512], start=False, stop=True)
                        nc.vector.tensor_copy(k_sb[:, mc, nb * 512:(nb + 1) * 512], ps)
                # vT[n, c] = (kv^T @ vw^T)[n, c] + vb[c], written per-head with
                # a ones column appended (softmax denominator trick)
                for nn in range(24):
                    half, nnh = nn // 12, nn % 12
                    ps = proj_ps.tile([128, C], f32, tag="proj")
                    for kc2 in range(2):
                        nc.tensor.matmul(
                            ps, lhsT=kv_half[half][:, kc2, nnh * 128:(nnh + 1) * 128],
                            rhs=w_sb["vwT"][:, kc2, :], start=(kc2 == 0), stop=False)
                    nc.tensor.matmul(ps, lhsT=ones_row[0:1, 0:128], rhs=vb_row[:],
                                     start=False, stop=True)
                    nc.vector.tensor_copy(
                        vT_aug[:, nn, :, 0:D],
                        ps.rearrange("p (h e) -> p h e", h=NH))

            # ---- attention + o-proj + LN ----
            with tc.tile_pool(name="s_ps", bufs=3, space="PSUM") as s_pool, \
                 tc.tile_pool(name="o_ps", bufs=1, space="PSUM") as o_pool, \
                 tc.tile_pool(name="exps", bufs=3) as exp_pool, \
                 tc.tile_pool(name="tails", bufs=2) as tails, \
                 tc.tile_pool(name="norms", bufs=2) as norms, \
                 tc.tile_pool(name="fins", bufs=2) as fins:
                onrm_saved = []
                for qb in range(2):
                    # onrm[hg] accumulates the 4 normalized heads of chunk hg
                    onrm0 = norms.tile([128, 512], bf16, tag="onrm0")
                    onrm1 = norms.tile([128, 512], bf16, tag="onrm1")
                    onrm_tiles = [onrm0, onrm1]
                    onrm_saved.append(onrm_tiles)
                    for hp in range(4):          # head pairs
                        hg, sub = hp // 2, hp % 2
                        # the two heads of the pair accumulate CONCURRENTLY:
                        # col-groups 0-1 (partitions 0-32) and 2-3 (64-96)
                        po = o_pool.tile([128, 512], f32, tag="opo")
                        for kc in range(24):
                            ps = s_pool.tile([128, 2, 512], f32, tag="S")
                            for j in range(2):
                                pof = 64 * sub + 32 * j
                                nc.tensor.matmul(
                                    ps[:, j, :],
                                    lhsT=k_sb[pof:pof + 32, hg, kc * 128:(kc + 1) * 128],
                                    rhs=q_sb[pof:pof + 32, hg, qb * 512:(qb + 1) * 512],
                                    start=True, stop=True, tile_position=(pof, 0))
                            slot = (qb * 4 + hp) * 24 + kc
                            es = exp_pool.tile([128, 2, 512], bf16, tag="exp")
                            if _use_dve_exp(slot):
                                es_i = es.bitcast(i16)
                                nc.vector.tensor_scalar(
                                    out=es_i, in0=ps, scalar1=_SCHR_A16,
                                    scalar2=_SCHR_B16, op0=Alu.mult, op1=Alu.add)
                            else:
                                nc.scalar.activation(es, ps, Act.Exp, scale=SCALE)
                            for j in range(2):
                                nc.tensor.matmul(
                                    po[64 * j:64 * j + D + 1, :],
                                    lhsT=vT_aug[:, kc, hp * 2 + j, :],
                                    rhs=es[:, j, :],
                                    start=(kc == 0), stop=(kc == 23),
                                    tile_position=(0, 64 * j))
                        if True:
                            # tail: numerators + softmax denominators.  One copy
                            # moves both heads (lanes parallel; cost = 512 cols)
                            raw = tails.tile([128, 512], f32, tag="raw")
                            nc.vector.tensor_copy(raw, po)
                            # denominators live on ONE partition row per head;
                            # iterative reciprocal is 8cyc/elem/lane, so spread
                            # 1024 values over 32 partitions via DMA, recip, pack
                            dp = tails.tile([32, 32], f32, tag="dp")
                            for j in range(2):
                                nc.sync.dma_start(
                                    out=dp[16 * j:16 * j + 16, :],
                                    in_=raw[64 * j + D:64 * j + D + 1, :])
                            rp = tails.tile([32, 32], f32, tag="rp")
                            nc.vector.reciprocal(rp, dp)
                            rec = tails.tile([1, 2, 512], f32, tag="rec")
                            nc.sync.dma_start(
                                out=rec.rearrange("p a q -> p (a q)"), in_=rp)
                            oin = tails.tile([128, 512], f32, tag="oin")
                            rbc = tails.tile([128, 512], f32, tag="rbc")
                            for j in range(2):
                                pof = 64 * sub + 32 * j
                                nc.sync.dma_start(out=oin[pof:pof + 32, :],
                                                  in_=raw[64 * j:64 * j + D, :])
                                nc.sync.dma_start(out=rbc[pof:pof + 32, :],
                                                  in_=bcast_sbuf_row(rec[0:1, j, :], 32))
                            nc.gpsimd.tensor_mul(
                                onrm_tiles[hg][64 * sub:64 * sub + 64, :],
                                oin[64 * sub:64 * sub + 64, :],
                                rbc[64 * sub:64 * sub + 64, :])
                # o-projection + residual + bias + LayerNorm per 128 queries,
                # emitted AFTER both query-blocks' attention so the PE (in-order)
                # never stalls waiting for a softmax tail mid-stream
                for qb in range(2):
                    onrm_tiles = onrm_saved[qb]
                    for qc2 in range(4):
                        qoff = qb * 512 + qc2 * 128
                        pso = s_pool.tile([128, C], f32, tag="S")
                        for hgc in range(2):
                            nc.tensor.matmul(
                                pso, lhsT=onrm_tiles[hgc][:, qc2 * 128:(qc2 + 1) * 128],
                                rhs=w_sb["owT"][:, hgc, :],
                                start=(hgc == 0), stop=False)
                        for cc in range(2):
                            nc.tensor.matmul(
                                pso, lhsT=x_sb[:, cc, qoff:qoff + 128],
                                rhs=w_sb["ident"][:, cc, :], start=False, stop=False)
                        nc.tensor.matmul(pso, lhsT=ones_row[0:1, 0:128], rhs=ob_row[:],
                                         start=False, stop=True)
                        stats = fins.tile([128, 6], f32, tag="stats")
                        nc.vector.bn_stats(stats, pso)
                        mv = fins.tile([128, 2], f32, tag="mv")
                        nc.vector.bn_aggr(mv, stats)
                        # rstd = exp(-0.5*ln(var+eps)): stays in the same ACT
                        # table set as the softmax exp (no table reload)
                        lnv = fins.tile([128, 1], f32, tag="lnv")
                        nc.scalar.activation(lnv, mv[:, 1:2], Act.Ln, bias=eps_col[:, 0:1])
                        rstd = fins.tile([128, 1], f32, tag="rstd")
                        nc.scalar.activation(rstd, lnv, Act.Exp, scale=-0.5)
                        t1 = fins.tile([128, C], f32, tag="t1")
                        nc.vector.tensor_scalar(
                            out=t1, in0=pso, scalar1=mv[:, 0:1], scalar2=rstd,
                            op0=Alu.subtract, op1=Alu.mult)
                        t2 = fins.tile([128, C], f32, tag="t2")
                        nc.gpsimd.tensor_mul(t2, t1, lnw_bc)
                        t3 = fins.tile([128, C], f32, tag="t3")
                        nc.gpsimd.tensor_add(t3, t2, lnb_bc)
                        nc.sync.dma_start(out=y_d[qoff:qoff + 128, :], in_=t3)
    return nc


_CACHE = {}


def _get_program():
    if "nc" not in _CACHE:
        _apply_walrus_wait_patch()
        _CACHE["nc"] = build_program()
    return _CACHE["nc"]


def _make_in_maps(inputs):
    s3 = np.ascontiguousarray(np.asarray(inputs["s3"], dtype=np.float32))
    s4 = np.ascontiguousarray(np.asarray(inputs["s4"], dtype=np.float32))
    s5 = np.ascontiguousarray(np.asarray(inputs["s5"], dtype=np.float32))
    B = s3.shape[0]
    wts = {}
    for nm in ("qw", "kw", "vw", "ow"):
        wts[nm + "T"] = np.ascontiguousarray(
            np.asarray(inputs[nm], dtype=np.float32).T.astype(BF16))
    ident = np.eye(C, dtype=BF16)
    rows = {}
    for nm in ("qb", "kb", "vb", "ob"):
        rows[nm] = np.ascontiguousarray(
            np.asarray(inputs[nm], dtype=np.float32).reshape(1, C).astype(BF16))
    for nm in ("ln_w", "ln_b"):
        rows[nm] = np.ascontiguousarray(
            np.asarray(inputs[nm], dtype=np.float32).reshape(1, C))
    in_maps = []
    for core in range(N_CORES):
        b, qc = core // 4, core % 4
        x = np.ascontiguousarray(
            s3[b].reshape(C, -1)[:, qc * NQ_CORE:(qc + 1) * NQ_CORE].astype(BF16))
        kv = np.ascontiguousarray(np.concatenate(
            [s4[b].reshape(C, -1), s5[b].reshape(C, -1)], axis=1).astype(BF16))
        in_maps.append({
            "x": x, "kv": kv,
            "qwT": wts["qwT"], "kwT": wts["kwT"], "vwT": wts["vwT"],
            "owT": wts["owT"], "ident": ident,
            "qb2": rows["qb"], "kb2": rows["kb"], "vb2": rows["vb"],
            "ob2": rows["ob"], "lnw2": rows["ln_w"], "lnb2": rows["ln_b"],
        })
    return in_maps


def _assemble(results, like):
    B, _, H, W = 2, C, 64, 64
    out = np.empty((B, C, H * W), dtype=np.float32)
    for core in range(N_CORES):
        b, qc = core // 4, core % 4
        out[b, :, qc * NQ_CORE:(qc + 1) * NQ_CORE] = results[core]["y"].T
    return out.reshape(B, C, H, W)


def kernel(**inputs):
    from concourse import bass2jax
    nc = _get_program()
    in_maps = _make_in_maps(inputs)
    results = bass2jax.run_bass_via_pjrt(nc, in_maps, n_cores=N_CORES)
    return _assemble(results, inputs["s3"])


# revision 12
# speedup vs baseline: 262.6194x; 1.0056x over previous
"""Content-guided attention kernel for Trainium2, 8 NeuronCores SPMD.

Sharding: 8 cores = (batch b in {0,1}) x (query-chunk qc in {0..3}).
Each core computes 1024 query positions of batch b end-to-end:
q/k/vT projections, 8-head attention over all 3072 keys, o-projection,
residual and LayerNorm.  No collectives needed; host splits/concats.

Per-core layout highlights:
 - all matmul operands are bf16 (fp32 PSUM accumulation): fp32 matmuls
   run as 2 HW passes each, bf16 runs single-pass and enables FWL for
   the per-k-chunk score weight loads, roughly halving PE busy time
 - scores computed transposed S^T[kpos, qpos] so softmax sum folds into the
   attn@V matmul via a ones-column appended to V^T (no partition reductions)
 - head_dim=32 scores matmuls are packed 4-at-a-time into the PE's 32-row
   groups via tile_position (4x concurrency at K=32)
 - exp split between ScalarE (exact table exp -> bf16 out) and VectorE
   (Schraudolph bit-trick exp in int16 -> bitcast bf16, ~3% elementwise,
   cancels in softmax normalization)
 - LayerNorm rstd computed as exp(-0.5*ln(var+eps)) to stay inside the
   single natural_log_exp ACT table set (no table switch thrash)
"""

import numpy as np
import ml_dtypes

BF16 = ml_dtypes.bfloat16

C = 256
NH = 8
D = 32
NQ_CORE = 1024
NK = 3072
N_CORES = 8
SCALE = float(D) ** -0.5

# Schraudolph exp constants for int16/bfloat16 bits (validated offline:
# 3.3% max elem rel err on the observed score range; cancels in softmax).
_SCHR_A16 = float(np.float32(SCALE * (1 << 7) / np.log(2.0)))
_SCHR_B16 = float(np.float32(127.0 * (1 << 7) - 365000.0 / 65536.0))

# every 3rd exp slot goes to the vector engine to offload the ACT bottleneck
def _use_dve_exp(slot: int) -> bool:
    return slot % 3 == 2


def _apply_walrus_wait_patch():
    """This walrus build accepts only ONE sync-wait per instruction; split
    extra waits onto single-wait NoOps inserted before the instruction
    (same engine, same block => per-engine program order preserved)."""
    import orjson
    import concourse.bass_utils as bass_utils
    import concourse.bass2jax as bass2jax

    if getattr(bass_utils, "_ant_wait_split_patch", False):
        return
    bass_utils._ant_wait_split_patch = True
    counter = [0]

    def _split_waits(bir_bytes: bytes) -> bytes:
        d = orjson.loads(bir_bytes)
        changed = False

        def process_blocks(blocks):
            nonlocal changed
            for b in blocks:
                insts = b.get("instructions")
                if insts:
                    new = []
                    for ins in insts:
                        si = ins.get("sync_info")
                        waits = si.get("on_wait") if si else None
                        if waits and len(waits) > 1:
                            changed = True
                            for w in waits[:-1]:
                                counter[0] += 1
                                new.append({
                                    "debug": ins.get("debug", 0),
                                    "engine": ins["engine"],
                                    "ins": [],
                                    "outs": [],
                                    "name": f"antwsplit-{counter[0]}",
                                    "opcode": "NoOp",
                                    "sync_info": {"on_wait": [w], "on_update": []},
                                })
                            si["on_wait"] = [waits[-1]]
                        new.append(ins)
                    b["instructions"] = new
                if b.get("blocks"):
                    process_blocks(b["blocks"])

        for f in d.get("functions", []):
            process_blocks(f.get("blocks", []))
        return orjson.dumps(d) if changed else bir_bytes

    orig = bass_utils.compile_bir_kernel

    def compile_bir_kernel(bir, tmpdir, neff_name="file.neff", **kw):
        if isinstance(bir, (bytes, bytearray)):
            bir = _split_waits(bytes(bir))
        elif isinstance(bir, str):
            bir = _split_waits(bir.encode()).decode()
        return orig(bir, tmpdir, neff_name=neff_name, **kw)

    bass_utils.compile_bir_kernel = compile_bir_kernel
    bass2jax.compile_bir_kernel = compile_bir_kernel


def build_program():
    import concourse.bass as bass
    import concourse.tile as tile
    from concourse import mybir

    f32 = mybir.dt.float32
    bf16 = mybir.dt.bfloat16
    i16 = mybir.dt.int16
    Alu = mybir.AluOpType
    Act = mybir.ActivationFunctionType

    nc = bass.Bass()

    x_d = nc.dram_tensor("x", [C, NQ_CORE], bf16, kind="ExternalInput")
    kv_d = nc.dram_tensor("kv", [C, NK], bf16, kind="ExternalInput")
    qwT_d = nc.dram_tensor("qwT", [C, C], bf16, kind="ExternalInput")
    kwT_d = nc.dram_tensor("kwT", [C, C], bf16, kind="ExternalInput")
    vwT_d = nc.dram_tensor("vwT", [C, C], bf16, kind="ExternalInput")
    owT_d = nc.dram_tensor("owT", [C, C], bf16, kind="ExternalInput")
    ident_d = nc.dram_tensor("ident", [C, C], bf16, kind="ExternalInput")
    qb_d = nc.dram_tensor("qb2", [1, C], bf16, kind="ExternalInput")
    kb_d = nc.dram_tensor("kb2", [1, C], bf16, kind="ExternalInput")
    vb_d = nc.dram_tensor("vb2", [1, C], bf16, kind="ExternalInput")
    ob_d = nc.dram_tensor("ob2", [1, C], bf16, kind="ExternalInput")
    lnw_d = nc.dram_tensor("lnw2", [1, C], f32, kind="ExternalInput")
    lnb_d = nc.dram_tensor("lnb2", [1, C], f32, kind="ExternalInput")
    y_d = nc.dram_tensor("y", [NQ_CORE, C], f32, kind="ExternalOutput")

    def bcast_part(ap, n):
        # partition-stride-0 view: replicate one partition row across n
        # (DRAM sources only; SBUF partition dims need nonzero step)
        return bass.AP(tensor=ap.tensor, offset=ap.offset,
                       ap=[[0, n]] + [list(a) for a in ap.ap[1:]])

    def bcast_sbuf_row(ap, n):
        # SBUF [1, F] row -> [n, F] DMA source: keep the 1-partition dim,
        # replicate via a step-0 free dim (legal for DMA reads)
        return bass.AP(tensor=ap.tensor, offset=ap.offset,
                       ap=[list(ap.ap[0]), [0, n]] + [list(a) for a in ap.ap[1:]])

    from contextlib import ExitStack
    with tile.TileContext(nc) as tc, ExitStack() as ctx:
            consts = ctx.enter_context(tc.tile_pool(name="consts", bufs=1))
            data = ctx.enter_context(tc.tile_pool(name="data", bufs=1))
            acts = ctx.enter_context(tc.tile_pool(name="acts", bufs=1))
            # ---- constants ----
            w_sb = {}
            for nm, dt_ in (("qwT", qwT_d), ("kwT", kwT_d), ("vwT", vwT_d),
                            ("owT", owT_d), ("ident", ident_d)):
                t = consts.tile([128, 2, C], bf16, tag=f"w_{nm}")
                nc.sync.dma_start(out=t, in_=dt_.rearrange("(a p) c -> p a c", p=128))
                w_sb[nm] = t
            qb_row = consts.tile([1, C], bf16, tag="qb_row")
            kb_row = consts.tile([1, C], bf16, tag="kb_row")
            vb_row = consts.tile([1, C], bf16, tag="vb_row")
            ob_row = consts.tile([1, C], bf16, tag="ob_row")
            for t, dt_ in ((qb_row, qb_d), (kb_row, kb_d), (vb_row, vb_d), (ob_row, ob_d)):
                nc.sync.dma_start(out=t, in_=dt_[:])
            lnw_bc = consts.tile([128, C], f32, tag="lnw_bc")
            lnb_bc = consts.tile([128, C], f32, tag="lnb_bc")
            nc.sync.dma_start(out=lnw_bc, in_=bcast_part(lnw_d[:], 128))
            nc.sync.dma_start(out=lnb_bc, in_=bcast_part(lnb_d[:], 128))
            ones_row = consts.tile([1, 512], bf16, tag="ones_row")
            nc.vector.memset(ones_row, 1.0)
            eps_col = consts.tile([128, 1], f32, tag="eps_col")
            nc.vector.memset(eps_col, 1e-5)

            # ---- input activations ----
            # x first (q-proj can start on it immediately); kv in halves so
            # k/v-proj start before the full tensor lands
            x_sb = data.tile([128, 2, NQ_CORE], bf16, tag="x_sb")
            nc.sync.dma_start(out=x_sb, in_=x_d.rearrange("(a p) n -> p a n", p=128))
            kv_half = []
            for h in range(2):
                t = data.tile([128, 2, NK // 2], bf16, tag=f"kv_sb{h}")
                nc.sync.dma_start(
                    out=t,
                    in_=kv_d[:, h * (NK // 2):(h + 1) * (NK // 2)]
                        .rearrange("(a p) n -> p a n", p=128))
                kv_half.append(t)

            q_sb = acts.tile([128, 2, NQ_CORE], bf16, tag="q_sb")
            k_sb = acts.tile([128, 2, NK], bf16, tag="k_sb")
            vT_aug = acts.tile([128, 24, NH, D + 1], bf16, tag="vT_aug")
            nc.vector.memset(vT_aug[:, :, :, D:D + 1], 1.0)
            # partition-rotated (by 64) copies of q/k: alternate score matmuls
            # between PE row-groups {0,32} and {64,96} so consecutive k-chunks'
            # weight loads and matmuls overlap instead of serializing
            q_shift = acts.tile([128, 2, NQ_CORE], bf16, tag="q_shift")
            k_shift = acts.tile([128, 2, NK], bf16, tag="k_shift")

            # ---- projections ----
            with tc.tile_pool(name="proj_ps", bufs=4, space="PSUM") as proj_ps:
                # q = qw @ x + qb   (chunks of output channels x 512 cols)
                for mc in range(2):
                    for nb in range(2):
                        ps = proj_ps.tile([128, 512], f32, tag="proj")
                        for kc2 in range(2):
                            nc.tensor.matmul(
                                ps, lhsT=w_sb["qwT"][:, kc2, mc * 128:(mc + 1) * 128],
                                rhs=x_sb[:, kc2, nb * 512:(nb + 1) * 512],
                                start=(kc2 == 0), stop=False)
                        nc.tensor.matmul(
                            ps, lhsT=qb_row[:, mc * 128:(mc + 1) * 128],
                            rhs=ones_row[:, 0:512], start=False, stop=True)
                        nc.vector.tensor_copy(q_sb[:, mc, nb * 512:(nb + 1) * 512], ps)
                # k = kw @ kv + kb
                for mc in range(2):
                    for nb in range(6):
                        half, nbh = nb // 3, nb % 3
                        ps = proj_ps.tile([128, 512], f32, tag="proj")
                        for kc2 in range(2):
                            nc.tensor.matmul(
                                ps, lhsT=w_sb["kwT"][:, kc2, mc * 128:(mc + 1) * 128],
                                rhs=kv_half[half][:, kc2, nbh * 512:(nbh + 1) * 512],
                                start=(kc2 == 0), stop=False)
                        nc.tensor.matmul(
                            ps, lhsT=kb_row[:, mc * 128:(mc + 1) * 128],
                            rhs=ones_row[:, 0:512], start=False, stop=True)
                        nc.vector.tensor_copy(k_sb[:, mc, nb * 512:(nb + 1) * 512], ps)
                # vT[n, c] = (kv^T @ vw^T)[n, c] + vb[c], written per-head with
                # a ones column appended (softmax denominator trick)
                for nn in range(24):
                    half, nnh = nn // 12, nn % 12
                    ps = proj_ps.tile([128, C], f32, tag="proj")
                    for kc2 in range(2):
                        nc.tensor.matmul(
                            ps, lhsT=kv_half[half][:, kc2, nnh * 128:(nnh + 1) * 128],
                            rhs=w_sb["vwT"][:, kc2, :], start=(kc2 == 0), stop=False)
                    nc.tensor.matmul(ps, lhsT=ones_row[0:1, 0:128], rhs=vb_row[:],
                                     start=False, stop=True)
                    nc.vector.tensor_copy(
                        vT_aug[:, nn, :, 0:D],
                        ps.rearrange("p (h e) -> p h e", h=NH))
                # build the rotated q/k copies once the projections land
                for t_dst, t_src in ((q_shift, q_sb), (k_shift, k_sb)):
                    nc.sync.dma_start(out=t_dst[0:64, :, :], in_=t_src[64:128, :, :])
                    nc.sync.dma_start(out=t_dst[64:128, :, :], in_=t_src[0:64, :, :])

            # ---- attention + o-proj + LN ----
            with tc.tile_pool(name="s_ps", bufs=3, space="PSUM") as s_pool, \
                 tc.tile_pool(name="o_ps", bufs=1, space="PSUM") as o_pool, \
                 tc.tile_pool(name="exps", bufs=3) as exp_pool, \
                 tc.tile_pool(name="tails", bufs=2) as tails, \
                 tc.tile_pool(name="norms", bufs=2) as norms, \
                 tc.tile_pool(name="fins", bufs=2) as fins:
                onrm_saved = []
                for qb in range(2):
                    # onrm[hg] accumulates the 4 normalized heads of chunk hg
                    onrm0 = norms.tile([128, 512], bf16, tag="onrm0")
                    onrm1 = norms.tile([128, 512], bf16, tag="onrm1")
                    onrm_tiles = [onrm0, onrm1]
                    onrm_saved.append(onrm_tiles)
                    for hp in range(4):          # head pairs
                        hg, sub = hp // 2, hp % 2
                        # the two heads of the pair accumulate CONCURRENTLY:
                        # col-groups 0-1 (partitions 0-32) and 2-3 (64-96)
                        po = o_pool.tile([128, 512], f32, tag="opo")
                        # kc blocks of 3 (matching the 3 psum bufs): the 3
                        # score-pairs run back-to-back (alternate row-groups via
                        # the rotated q/k copies => they overlap on the PE), then
                        # the 3 V-pairs; exp of one block hides under the next
                        for kc0 in range(0, 24, 3):
                            pss, ess = [], []
                            for e in range(3):
                                kc = kc0 + e
                                sh = e % 2
                                ps = s_pool.tile([128, 2, 512], f32, tag="S")
                                kt = k_shift if sh else k_sb
                                qt = q_shift if sh else q_sb
                                for j in range(2):
                                    pof = (64 * sub + 32 * j + 64 * sh) % 128
                                    nc.tensor.matmul(
                                        ps[:, j, :],
                                        lhsT=kt[pof:pof + 32, hg, kc * 128:(kc + 1) * 128],
                                        rhs=qt[pof:pof + 32, hg, qb * 512:(qb + 1) * 512],
                                        start=True, stop=True, tile_position=(pof, 0))
                                pss.append(ps)
                            for e in range(3):
                                kc = kc0 + e
                                slot = (qb * 4 + hp) * 24 + kc
                                es = exp_pool.tile([128, 2, 512], bf16, tag="exp")
                                if _use_dve_exp(slot):
                                    es_i = es.bitcast(i16)
                                    nc.vector.tensor_scalar(
                                        out=es_i, in0=pss[e], scalar1=_SCHR_A16,
                                        scalar2=_SCHR_B16, op0=Alu.mult, op1=Alu.add)
                                else:
                                    nc.scalar.activation(es, pss[e], Act.Exp, scale=SCALE)
                                ess.append(es)
                            for e in range(3):
                                kc = kc0 + e
                                for j in range(2):
                                    nc.tensor.matmul(
                                        po[64 * j:64 * j + D + 1, :],
                                        lhsT=vT_aug[:, kc, hp * 2 + j, :],
                                        rhs=ess[e][:, j, :],
                                        start=(kc == 0), stop=(kc == 23),
                                        tile_position=(0, 64 * j))
                        if True:
                            # tail: numerators + softmax denominators.  One copy
                            # moves both heads (lanes parallel; cost = 512 cols)
                            raw = tails.tile([128, 512], f32, tag="raw")
                            nc.vector.tensor_copy(raw, po)
                            # denominators live on ONE partition row per head;
                            # iterative reciprocal is 8cyc/elem/lane, so spread
                            # 1024 values over 32 partitions via DMA, recip, pack
                            dp = tails.tile([32, 32], f32, tag="dp")
                            for j in range(2):
                                nc.sync.dma_start(
                                    out=dp[16 * j:16 * j + 16, :],
                                    in_=raw[64 * j + D:64 * j + D + 1, :])
                            rp = tails.tile([32, 32], f32, tag="rp")
                            nc.vector.reciprocal(rp, dp)
                            rec = tails.tile([1, 2, 512], f32, tag="rec")
                            nc.sync.dma_start(
                                out=rec.rearrange("p a q -> p (a q)"), in_=rp)
                            oin = tails.tile([128, 512], f32, tag="oin")
                            rbc = tails.tile([128, 512], f32, tag="rbc")
                            for j in range(2):
                                pof = 64 * sub + 32 * j
                                nc.sync.dma_start(out=oin[pof:pof + 32, :],
                                                  in_=raw[64 * j:64 * j + D, :])
                                nc.sync.dma_start(out=rbc[pof:pof + 32, :],
                                                  in_=bcast_sbuf_row(rec[0:1, j, :], 32))
                            nc.gpsimd.tensor_mul(
                                onrm_tiles[hg][64 * sub:64 * sub + 64, :],
                                oin[64 * sub:64 * sub + 64, :],
                                rbc[64 * sub:64 * sub + 64, :])
                # o-projection + residual + bias + LayerNorm per 128 queries,
                # emitted AFTER both query-blocks' attention so the PE (in-order)
                # never stalls waiting for a softmax tail mid-stream
                for qb in range(2):
                    onrm_tiles = onrm_saved[qb]
                    for qc2 in range(4):
                        qoff = qb * 512 + qc2 * 128
                        pso = s_pool.tile([128, C], f32, tag="S")
                        for hgc in range(2):
                            nc.tensor.matmul(
                                pso, lhsT=onrm_tiles[hgc][:, qc2 * 128:(qc2 + 1) * 128],
                                rhs=w_sb["owT"][:, hgc, :],
                                start=(hgc == 0), stop=False)
                        for cc in range(2):
                            nc.tensor.matmul(
                                pso, lhsT=x_sb[:, cc, qoff:qoff + 128],
                                rhs=w_sb["ident"][:, cc, :], start=False, stop=False)
                        nc.tensor.matmul(pso, lhsT=ones_row[0:1, 0:128], rhs=ob_row[:],
                                         start=False, stop=True)
                        stats = fins.tile([128, 6], f32, tag="stats")
                        nc.vector.bn_stats(stats, pso)
                        mv = fins.tile([128, 2], f32, tag="mv")
                        nc.vector.bn_aggr(mv, stats)
                        # rstd = exp(-0.5*ln(var+eps)): stays in the same ACT
                        # table set as the softmax exp (no table reload)
                        lnv = fins.tile([128, 1], f32, tag="lnv")
                        nc.scalar.activation(lnv, mv[:, 1:2], Act.Ln, bias=eps_col[:, 0:1])
                        rstd = fins.tile([128, 1], f32, tag="rstd")
                        nc.scalar.activation(rstd, lnv, Act.Exp, scale=-0.5)
                        t1 = fins.tile([128, C], f32, tag="t1")
                        nc.vector.tensor_scalar(
                            out=t1, in0=pso, scalar1=mv[:, 0:1], scalar2=rstd,
                            op0=Alu.subtract, op1=Alu.mult)
                        t2 = fins.tile([128, C], f32, tag="t2")
                        nc.gpsimd.tensor_mul(t2, t1, lnw_bc)
                        t3 = fins.tile([128, C], f32, tag="t3")
                        nc.gpsimd.tensor_add(t3, t2, lnb_bc)
                        nc.sync.dma_start(out=y_d[qoff:qoff + 128, :], in_=t3)
    return nc


_CACHE = {}


def _get_program():
    if "nc" not in _CACHE:
        _apply_walrus_wait_patch()
        _CACHE["nc"] = build_program()
    return _CACHE["nc"]


def _make_in_maps(inputs):
    s3 = np.ascontiguousarray(np.asarray(inputs["s3"], dtype=np.float32))
    s4 = np.ascontiguousarray(np.asarray(inputs["s4"], dtype=np.float32))
    s5 = np.ascontiguousarray(np.asarray(inputs["s5"], dtype=np.float32))
    B = s3.shape[0]
    wts = {}
    for nm in ("qw", "kw", "vw", "ow"):
        wts[nm + "T"] = np.ascontiguousarray(
            np.asarray(inputs[nm], dtype=np.float32).T.astype(BF16))
    ident = np.eye(C, dtype=BF16)
    rows = {}
    for nm in ("qb", "kb", "vb", "ob"):
        rows[nm] = np.ascontiguousarray(
            np.asarray(inputs[nm], dtype=np.float32).reshape(1, C).astype(BF16))
    for nm in ("ln_w", "ln_b"):
        rows[nm] = np.ascontiguousarray(
            np.asarray(inputs[nm], dtype=np.float32).reshape(1, C))
    in_maps = []
    for core in range(N_CORES):
        b, qc = core // 4, core % 4
        x = np.ascontiguousarray(
            s3[b].reshape(C, -1)[:, qc * NQ_CORE:(qc + 1) * NQ_CORE].astype(BF16))
        kv = np.ascontiguousarray(np.concatenate(
            [s4[b].reshape(C, -1), s5[b].reshape(C, -1)], axis=1).astype(BF16))
        in_maps.append({
            "x": x, "kv": kv,
            "qwT": wts["qwT"], "kwT": wts["kwT"], "vwT": wts["vwT"],
            "owT": wts["owT"], "ident": ident,
            "qb2": rows["qb"], "kb2": rows["kb"], "vb2": rows["vb"],
            "ob2": rows["ob"], "lnw2": rows["ln_w"], "lnb2": rows["ln_b"],
        })
    return in_maps


def _assemble(results, like):
    B, _, H, W = 2, C, 64, 64
    out = np.empty((B, C, H * W), dtype=np.float32)
    for core in range(N_CORES):
        b, qc = core // 4, core % 4
        out[b, :, qc * NQ_CORE:(qc + 1) * NQ_CORE] = results[core]["y"].T
    return out.reshape(B, C, H, W)


def kernel(**inputs):
    from concourse import bass2jax
    nc = _get_program()
    in_maps = _make_in_maps(inputs)
    results = bass2jax.run_bass_via_pjrt(nc, in_maps, n_cores=N_CORES)
    return _assemble(results, inputs["s3"])
